# revision 1
# baseline (speedup 1.0000x reference)
"""Trainium2 Bass kernel for nn_LlamaAttention_61899068670751.

Sparse (streaming-LLM) attention layer, sharded tensor-parallel over heads
across 8 NeuronCores:
  - core c owns q-heads [4c..4c+3] and kv-head c (GQA group = 4)
  - QKV projections computed per-core with column-sharded weights
  - causal scores computed once per head; both softmax branches (full causal
    and sink+sliding-window) share exp(s) and are folded into a single PV
    matmul via per-row/per-region coefficients
  - the router MLP is computed redundantly on every core after a [128]
    feature AllReduce; its hard gate enters the coefficients
  - o is exchanged with an AllToAll so each core computes 256 rows of the
    final o @ Wo with the full head dimension; host concatenates row shards

All matmuls run as float32r (full-rate fp32 on the PE array).
"""
import numpy as np
from contextlib import ExitStack

import concourse.bacc as bacc
import concourse.mybir as mybir
import concourse.tile as tile
from concourse.bass_utils import run_bass_kernel_spmd

dt = mybir.dt
AF = mybir.ActivationFunctionType
ALU = mybir.AluOpType
AX = mybir.AxisListType

NCORES = 8
S, H, KV, D, HID = 2048, 32, 8, 128, 4096
SINK, WIN, POOL = 128, 1024, 100
HLOC = H // NCORES          # 4 q heads per core
NBLK = S // 128             # 16 row/col blocks
NCH = 4                     # s-chunks of 512
CH = 512
KT = HID // 128             # 32 contraction tiles
SCALE = 1.0 / float(np.sqrt(D))
NEG = -1.0e30
ROWS = S // NCORES          # 256 output rows per core


def _groups(I):
    """Right-aligned 4-block groups over causal blocks 0..I."""
    n = I + 1
    lo = n % 4
    g = [(0, lo)] if lo else []
    g += [(s, 4) for s in range(lo, n, 4)]
    return g


KNOBS = dict(phases=3, wo_bufs=24, w_bufs=8, hs_bufs=1, sc_bufs=3, pt_bufs=2,
             e_bufs=3, p_bufs=3, pT_bufs=2, o_bufs=1, tr_bufs=2)


def build():
    nc = bacc.Bacc("TRN2", target_bir_lowering=False, debug=False,
                   num_devices=NCORES)

    def din(name, shape, d=dt.float32r):
        return nc.dram_tensor(name, shape, d, kind="ExternalInput").ap()

    hs_d = din("hs", [S, HID])
    wqkv_d = din("wqkv", [HID, 768])
    wo_d = din("wo", [HID, HID])
    cos2_d = din("cos2", [128, S], dt.float32)
    sin2_d = din("sin2", [128, S], dt.float32)
    ident_d = din("ident", [128, 128])
    diagneg_d = din("diagneg", [128, 128], dt.float32)
    triup_d = din("triup", [128, 128], dt.float32)
    ones_d = din("ones", [1, 128], dt.float32)
    fe1_d = din("fe1", [128, 1024], dt.float32)
    fe2_d = din("fe2", [128, 8 * 256], dt.float32)
    r1_d = din("r1", [128, 2 * 512], dt.float32)
    r2_d = din("r2", [128, 4 * 128], dt.float32)
    r3_d = din("r3", [128, 1], dt.float32)
    b1_d = din("b1", [128, 8], dt.float32)
    b2_d = din("b2", [128, 2], dt.float32)
    rb1_d = din("rb1", [128, 4], dt.float32)
    rb2_d = din("rb2", [128, 1], dt.float32)
    rb3_d = din("rb3", [1, 1], dt.float32)
    noise_d = din("noise", [1, 1], dt.float32)
    eps_d = din("eps", [1, 1], dt.float32)

    out_d = nc.dram_tensor("out_rows", [ROWS, HID], dt.float32,
                           kind="ExternalOutput").ap()

    with tile.TileContext(nc) as tc, ExitStack() as top:
        # ---- long-lived pools -------------------------------------------
        const = top.enter_context(tc.tile_pool(name="const", bufs=1))
        persist = top.enter_context(tc.tile_pool(name="persist", bufs=1))
        dram = top.enter_context(tc.tile_pool(name="dram", bufs=1, space="DRAM"))

        ident = const.tile([128, 128], dt.float32r)
        diagneg = const.tile([128, 128], dt.float32)
        triup = const.tile([128, 128], dt.float32)
        ones_r = const.tile([1, 128], dt.float32)
        nc.sync.dma_start(ident[:], ident_d[:])
        nc.sync.dma_start(diagneg[:], diagneg_d[:])
        nc.sync.dma_start(triup[:], triup_d[:])
        nc.sync.dma_start(ones_r[:], ones_d[:])

        qT = [persist.tile([128, S], dt.float32r, name=f"qT{h}", tag=f"qT{h}")
              for h in range(HLOC)]
        kT = persist.tile([128, S], dt.float32r)
        vN = persist.tile([128, NBLK * 128], dt.float32r)   # v natural, per block

        # collective bounce buffers
        a2a_in0 = dram.tile([NCORES, 2 * 128, ROWS], dt.float32r)
        a2a_out0 = dram.tile([NCORES, 2 * 128, ROWS], dt.float32r)
        a2a_in1 = dram.tile([NCORES, 2 * 128, ROWS], dt.float32r)
        a2a_out1 = dram.tile([NCORES, 2 * 128, ROWS], dt.float32r)
        cc_in = dram.tile([128, 1], dt.float32)
        cc_out = dram.tile([128, 1], dt.float32, addr_space="Shared")

        # ---- phase 1: hs transpose + QKV projections + rope -------------
        with ExitStack() as ph1:
            p_hs = ph1.enter_context(tc.tile_pool(name="hs", bufs=KNOBS["hs_bufs"]))
            p_hsT = ph1.enter_context(tc.tile_pool(name="hsT", bufs=2))
            p_w = ph1.enter_context(tc.tile_pool(name="wslab", bufs=KNOBS["w_bufs"]))
            p_rope = ph1.enter_context(tc.tile_pool(name="rope", bufs=2))
            p_cs = ph1.enter_context(tc.tile_pool(name="cs", bufs=2))
            ps_tr = ph1.enter_context(
                tc.tile_pool(name="ps_tr", bufs=KNOBS["tr_bufs"], space="PSUM"))
            ps_acc = ph1.enter_context(
                tc.tile_pool(name="ps_acc", bufs=1, space="PSUM"))

            KH = KT // 2        # 16 k-tiles per half
            for g in range(NCH):
                s0 = g * CH
                accs = [ps_acc.tile([128, CH], dt.float32, tag=f"acc{i}",
                                    name=f"acc{i}")
                        for i in range(6)]
                for half in range(2):
                    k0 = half * KH
                    hsT = p_hsT.tile([128, KH * CH], dt.float32r, tag="hsT")
                    hsrows = []
                    for ss in range(4):
                        hsrow = p_hs.tile([128, KH * 128], dt.float32r,
                                          tag=f"hs{ss}", name=f"hs{ss}")
                        nc.sync.dma_start(
                            hsrow[:], hs_d[s0 + ss * 128: s0 + (ss + 1) * 128,
                                           k0 * 128:(k0 + KH) * 128])
                        hsrows.append(hsrow)
                    for kk in range(KH):
                        ptr = ps_tr.tile([128, 512], dt.float32r, tag="tr")
                        for ss in range(4):
                            nc.tensor.transpose(
                                ptr[:, ss * 128:(ss + 1) * 128],
                                hsrows[ss][:, kk * 128:(kk + 1) * 128],
                                ident[:])
                        nc.any.tensor_copy(
                            hsT[:, kk * CH:(kk + 1) * CH], ptr[:])
                    for kk in range(KH):
                        kt = k0 + kk
                        wsl = p_w.tile([128, 768], dt.float32r, tag="w")
                        nc.sync.dma_start(wsl[:],
                                          wqkv_d[kt * 128:(kt + 1) * 128, :])
                        for i in range(6):
                            nc.tensor.matmul(
                                accs[i][:], wsl[:, i * 128:(i + 1) * 128],
                                hsT[:, kk * CH:(kk + 1) * CH],
                                start=(kt == 0), stop=(kt == KT - 1))

                # rope for q heads (0..3) and k (4)
                cos_sl = p_cs.tile([128, CH], dt.float32, tag="cos")
                sin_sl = p_cs.tile([128, CH], dt.float32, tag="sin")
                nc.sync.dma_start(cos_sl[:], cos2_d[:, s0:s0 + CH])
                nc.sync.dma_start(sin_sl[:], sin2_d[:, s0:s0 + CH])
                for i in range(5):
                    dest = qT[i] if i < HLOC else kT
                    lin = p_rope.tile([128, CH], dt.float32, tag="lin")
                    rot = p_rope.tile([128, CH], dt.float32, tag="rot")
                    t1 = p_rope.tile([128, CH], dt.float32, tag="t1")
                    t2 = p_rope.tile([128, CH], dt.float32, tag="t2")
                    nc.scalar.copy(lin[:], accs[i][:])
                    nc.sync.dma_start(rot[0:64, :], lin[64:128, :])
                    nc.sync.dma_start(rot[64:128, :], lin[0:64, :])
                    nc.vector.tensor_tensor(t1[:], lin[:], cos_sl[:], ALU.mult)
                    nc.vector.tensor_tensor(t2[:], rot[:], sin_sl[:], ALU.mult)
                    nc.vector.tensor_tensor(dest[:, s0:s0 + CH], t1[:], t2[:],
                                            ALU.add)
                # v: copy then transpose to natural layout
                vT = p_rope.tile([128, CH], dt.float32r, tag="vT")
                nc.scalar.copy(vT[:], accs[5][:])
                for ss in range(4):
                    ptr = ps_tr.tile([128, 128], dt.float32r, tag="tr")
                    nc.tensor.transpose(ptr[:], vT[:, ss * 128:(ss + 1) * 128],
                                        ident[:])
                    nc.any.tensor_copy(
                        vN[:, (g * 4 + ss) * 128:(g * 4 + ss + 1) * 128], ptr[:])

        # ---- phase 2: router + attention --------------------------------
        with ExitStack() as ph2:
          if KNOBS["phases"] >= 2:
              p_mlp = ph2.enter_context(tc.tile_pool(name="mlp", bufs=1))
              p_e = ph2.enter_context(tc.tile_pool(name="eband", bufs=KNOBS["e_bufs"]))
              p_p = ph2.enter_context(tc.tile_pool(name="pband", bufs=KNOBS["p_bufs"]))
              p_pT = ph2.enter_context(tc.tile_pool(name="pT", bufs=KNOBS["pT_bufs"]))
              p_sm = ph2.enter_context(tc.tile_pool(name="sums", bufs=4))
              p_ob = ph2.enter_context(tc.tile_pool(name="obuf", bufs=2))
              ps_sc = ph2.enter_context(
                  tc.tile_pool(name="ps_sc", bufs=KNOBS["sc_bufs"], space="PSUM"))
              ps_pt = ph2.enter_context(
                  tc.tile_pool(name="ps_pt", bufs=KNOBS["pt_bufs"], space="PSUM"))
              ps_o = ph2.enter_context(
                  tc.tile_pool(name="ps_o", bufs=KNOBS["o_bufs"], space="PSUM"))
              mlp_ctx = ExitStack()
              ps_m = mlp_ctx.enter_context(
                  tc.tile_pool(name="ps_m", bufs=1, space="PSUM"))

              # --- router ---
              feat_acc = p_mlp.tile([128, 8], dt.float32)
              for h in range(HLOC):
                  nc.vector.tensor_reduce(feat_acc[:, h:h + 1],
                                          qT[h][:, 0:POOL], AX.X, ALU.add)
                  nc.vector.tensor_reduce(feat_acc[:, 4 + h:5 + h],
                                          qT[h][:, S - POOL:S], AX.X, ALU.add)
              feat_s = p_mlp.tile([128, 1], dt.float32)
              nc.vector.tensor_reduce(feat_s[:], feat_acc[:], AX.X, ALU.add)
              feat_r = p_mlp.tile([128, 1], dt.float32)
              nc.scalar.activation(feat_r[:], feat_s[:], AF.Copy,
                                   scale=1.0 / (2 * POOL * H))
              nc.sync.dma_start(cc_in[:], feat_r[:])
              nc.gpsimd.collective_compute(
                  "AllReduce", ALU.add,
                  replica_groups=[list(range(NCORES))],
                  ins=[cc_in.opt()], outs=[cc_out.opt()])
              featg = p_mlp.tile([128, 1], dt.float32)
              nc.sync.dma_start(featg[:], cc_out[:])

              # MLP weights
              fe1 = p_mlp.tile([128, 1024], dt.float32)
              fe2 = p_mlp.tile([128, 8 * 256], dt.float32)
              r1w = p_mlp.tile([128, 2 * 512], dt.float32)
              r2w = p_mlp.tile([128, 4 * 128], dt.float32)
              r3w = p_mlp.tile([128, 1], dt.float32)
              b1 = p_mlp.tile([128, 8], dt.float32)
              b2 = p_mlp.tile([128, 2], dt.float32)
              rb1 = p_mlp.tile([128, 4], dt.float32)
              rb2 = p_mlp.tile([128, 1], dt.float32)
              rb3 = p_mlp.tile([1, 1], dt.float32)
              noise = p_mlp.tile([1, 1], dt.float32)
              epsb = p_mlp.tile([1, 1], dt.float32)
              nc.sync.dma_start(epsb[:], eps_d[:])
              for t_, d_ in ((fe1, fe1_d), (fe2, fe2_d), (r1w, r1_d),
                             (r2w, r2_d), (r3w, r3_d), (b1, b1_d), (b2, b2_d),
                             (rb1, rb1_d), (rb2, rb2_d), (rb3, rb3_d),
                             (noise, noise_d)):
                  nc.sync.dma_start(t_[:], d_[:])

              def mlp_layer(vec_in, w_sb, ktiles, ntiles, bias, act, nwidth=128):
                  """vec_in: [128, ktiles] fp32r columns; returns [128, ntiles]."""
                  out_r = p_mlp.tile([128, max(ntiles, 1)], dt.float32,
                                     name=f"mlpv{len(mlp_tmp)}")
                  mlp_tmp.append(out_r)
                  ps = ps_m.tile([128, max(ntiles, 1)], dt.float32, tag="mlp",
                               name="mlpps")
                  for t in range(ntiles):
                      for k in range(ktiles):
                          nc.tensor.matmul(
                              ps[:, t:t + 1],
                              w_sb[:, (k * ntiles + t) * nwidth:
                                   (k * ntiles + t) * nwidth + nwidth],
                              vec_in[:, k:k + 1],
                              start=(k == 0), stop=(k == ktiles - 1))
                  for t in range(ntiles):
                      nc.scalar.activation(out_r[:, t:t + 1], ps[:, t:t + 1],
                                           act, bias=bias[:, t:t + 1])
                  return out_r

              mlp_tmp = []
              h1 = mlp_layer(featg, fe1, 1, 8, b1, AF.Silu)
              h2 = mlp_layer(h1, fe2, 8, 2, b2, AF.Identity)
              h3 = mlp_layer(h2, r1w, 2, 4, rb1, AF.Silu)
              h4 = mlp_layer(h3, r2w, 4, 1, rb2, AF.Silu)
              lps = ps_m.tile([1, 1], dt.float32, tag="mlp")
              nc.tensor.matmul(lps[:], r3w[:], h4[:], start=True, stop=True)
              logits = p_mlp.tile([1, 1], dt.float32)
              nc.scalar.activation(logits[:], lps[:], AF.Identity, bias=rb3[:])
              l1 = p_mlp.tile([1, 1], dt.float32)
              l2 = p_mlp.tile([1, 1], dt.float32)
              nc.scalar.activation(l1[:], noise[:], AF.Ln, bias=epsb[:])
              nc.scalar.activation(l2[:], l1[:], AF.Ln, bias=epsb[:], scale=-1.0)
              zin = p_mlp.tile([1, 1], dt.float32)
              nc.vector.tensor_tensor(zin[:], logits[:], l2[:], ALU.subtract)
              zsoft = p_mlp.tile([1, 1], dt.float32)
              nc.scalar.activation(zsoft[:], zin[:], AF.Sigmoid)
              zhard = p_mlp.tile([1, 1], dt.float32)
              nc.vector.tensor_scalar(zhard[:], zsoft[:], 0.5, None, ALU.is_gt)
              mps = ps_m.tile([128, 1], dt.float32, tag="mlp")
              nc.tensor.matmul(mps[:], ones_r[:], zhard[:], start=True, stop=True)
              mix = p_mlp.tile([128, 1], dt.float32)
              nc.scalar.copy(mix[:], mps[:])
              onem = p_mlp.tile([128, 1], dt.float32)
              nc.vector.tensor_scalar(onem[:], mix[:], -1.0, 1.0, ALU.mult,
                                      ALU.add)
              mlp_ctx.close()

              # --- attention ---
              for h in range(HLOC):
                  for g in range(NCH):
                      pT = p_pT.tile([128, NBLK * CH], dt.float32r, tag="pT")
                      for Ii in range(4):
                          I = 4 * g + Ii
                          nb_tot = (I + 1) * 128
                          e = p_e.tile([128, S], dt.float32, tag="e")
                          pband = p_p.tile([128, S], dt.float32r, tag="p")
                          sums = p_sm.tile([128, 16], dt.float32, tag="sums")
                          grps = _groups(I)
                          ng = len(grps)
                          for gi, (sb, nb) in enumerate(grps):
                              w = nb * 128
                              col = 4 - ng + gi
                              sc = ps_sc.tile([128, 512], dt.float32, tag="sc")
                              nc.tensor.matmul(
                                  sc[:, 0:w], qT[h][:, I * 128:(I + 1) * 128],
                                  kT[:, sb * 128: sb * 128 + w],
                                  start=True, stop=True)
                              if gi == ng - 1:
                                  nc.vector.tensor_tensor(
                                      sc[:, w - 128:w], sc[:, w - 128:w],
                                      diagneg[:], ALU.add)
                              nc.scalar.activation(
                                  e[:, sb * 128: sb * 128 + w], sc[:, 0:w],
                                  AF.Exp, scale=SCALE,
                                  accum_out=sums[:, col:col + 1])
                          if I >= 9:
                              tmask = p_sm.tile([128, 128], dt.float32,
                                                tag="tmask")
                              nc.vector.tensor_reduce(
                                  sums[:, 4:5], e[:, 0:128], AX.X, ALU.add)
                              nc.vector.tensor_tensor(
                                  tmask[:], e[:, (I - 8) * 128:(I - 7) * 128],
                                  triup[:], ALU.mult)
                              nc.vector.tensor_reduce(
                                  sums[:, 5:6], tmask[:], AX.X, ALU.add)
                              nc.vector.tensor_reduce(
                                  sums[:, 6:7], sums[:, 4 - ng:4], AX.X, ALU.add)
                              nc.vector.tensor_reduce(
                                  sums[:, 7:8], sums[:, 2:6], AX.X, ALU.add)
                              nc.vector.reciprocal(sums[:, 8:9], sums[:, 6:7])
                              nc.vector.reciprocal(sums[:, 9:10], sums[:, 7:8])
                              nc.vector.tensor_tensor(
                                  sums[:, 10:11], sums[:, 8:9], onem[:], ALU.mult)
                              nc.vector.tensor_tensor(
                                  sums[:, 11:12], sums[:, 9:10], mix[:], ALU.mult)
                              nc.vector.tensor_tensor(
                                  sums[:, 12:13], sums[:, 10:11], sums[:, 11:12],
                                  ALU.add)
                              a_ap = sums[:, 12:13]
                              b_ap = sums[:, 10:11]
                              amb_ap = sums[:, 11:12]
                              nc.vector.tensor_scalar(
                                  pband[:, 0:128], e[:, 0:128], a_ap, None,
                                  ALU.mult)
                              if I >= 10:
                                  nc.vector.tensor_scalar(
                                      pband[:, 128:(I - 8) * 128],
                                      e[:, 128:(I - 8) * 128], b_ap, None,
                                      ALU.mult)
                              nc.vector.tensor_scalar(
                                  pband[:, (I - 8) * 128:(I - 7) * 128],
                                  e[:, (I - 8) * 128:(I - 7) * 128], b_ap, None,
                                  ALU.mult)
                              nc.vector.scalar_tensor_tensor(
                                  pband[:, (I - 8) * 128:(I - 7) * 128],
                                  tmask[:], amb_ap,
                                  pband[:, (I - 8) * 128:(I - 7) * 128],
                                  ALU.mult, ALU.add)
                              nc.vector.tensor_scalar(
                                  pband[:, (I - 7) * 128:nb_tot],
                                  e[:, (I - 7) * 128:nb_tot], a_ap, None,
                                  ALU.mult)
                          else:
                              nc.vector.tensor_reduce(
                                  sums[:, 6:7], sums[:, 4 - ng:4], AX.X, ALU.add)
                              nc.vector.reciprocal(sums[:, 8:9], sums[:, 6:7])
                              nc.vector.tensor_scalar(
                                  pband[:, 0:nb_tot], e[:, 0:nb_tot],
                                  sums[:, 8:9], None, ALU.mult)
                          pT3 = pT[:, :].rearrange("p (J c) -> p J c", c=CH)
                          for J0 in range(0, I + 1, 4):
                              nb4 = min(4, I + 1 - J0)
                              ptp = ps_pt.tile([128, 512], dt.float32r, tag="pt")
                              for jj in range(nb4):
                                  nc.tensor.transpose(
                                      ptp[:, jj * 128:(jj + 1) * 128],
                                      pband[:, (J0 + jj) * 128:
                                            (J0 + jj + 1) * 128],
                                      ident[:])
                              nc.any.tensor_copy(
                                  pT3[:, J0:J0 + nb4,
                                      Ii * 128:(Ii + 1) * 128],
                                  ptp[:, 0:nb4 * 128].rearrange(
                                      "p (J c) -> p J c", c=128))
                      # PV for this (h, chunk)
                      ops = ps_o.tile([128, CH], dt.float32, tag="o")
                      last_J = 4 * g + 3
                      for J in range(last_J + 1):
                          k = J - 4 * g
                          if k <= 0:
                              nc.tensor.matmul(
                                  ops[:], vN[:, J * 128:(J + 1) * 128],
                                  pT[:, J * CH:(J + 1) * CH],
                                  start=(J == 0), stop=(J == last_J))
                          else:
                              nc.tensor.matmul(
                                  ops[:, k * 128:CH],
                                  vN[:, J * 128:(J + 1) * 128],
                                  pT[:, J * CH + k * 128:(J + 1) * CH],
                                  start=False, stop=(J == last_J))
                      osb = p_ob.tile([128, CH], dt.float32r, tag="osb")
                      nc.scalar.copy(osb[:], ops[:])
                      a2a_in_h = a2a_in0 if h < 2 else a2a_in1
                      hh = h % 2
                      nc.sync.dma_start(
                          a2a_in_h[2 * g, hh * 128:(hh + 1) * 128, :],
                          osb[:, 0:ROWS])
                      nc.sync.dma_start(
                          a2a_in_h[2 * g + 1, hh * 128:(hh + 1) * 128, :],
                          osb[:, ROWS:CH])
                  if h == 1:
                      nc.gpsimd.collective_compute(
                          "AllToAll", ALU.bypass,
                          replica_groups=[list(range(NCORES))],
                          ins=[a2a_in0.opt()], outs=[a2a_out0.opt()])
                  if h == 3:
                      nc.gpsimd.collective_compute(
                          "AllToAll", ALU.bypass,
                          replica_groups=[list(range(NCORES))],
                          ins=[a2a_in1.opt()], outs=[a2a_out1.opt()])

        # ---- phase 3: AllToAll + output projection ----------------------
        with ExitStack() as ph3:
          if KNOBS["phases"] >= 3:
              p_oT = ph3.enter_context(tc.tile_pool(name="oT", bufs=1))
              p_wo = ph3.enter_context(tc.tile_pool(name="wo", bufs=KNOBS["wo_bufs"]))
              p_os = ph3.enter_context(tc.tile_pool(name="outsb", bufs=2))
              ps_w = ph3.enter_context(
                  tc.tile_pool(name="ps_w", bufs=2, space="PSUM"))

              KT_ORDER = [4 * p + t for t in (0, 1, 2, 3) for p in range(NCORES)]
              oT = p_oT.tile([128, KT * ROWS], dt.float32r)
              for kt in KT_ORDER:
                  p, t = kt // HLOC, kt % HLOC
                  src = a2a_out0 if t < 2 else a2a_out1
                  nc.sync.dma_start(
                      oT[:, kt * ROWS:(kt + 1) * ROWS],
                      src[p, (t % 2) * 128:(t % 2 + 1) * 128, :])

              for ngi in range(8):
                  pso = [ps_w.tile([128, 512], dt.float32, tag=f"wo{st}",
                                  name=f"wo{st}")
                         for st in range(2)]
                  for ki, kt in enumerate(KT_ORDER):
                      wsl = p_wo.tile([128, 512], dt.float32r, tag="wo")
                      nc.sync.dma_start(
                          wsl[:], wo_d[kt * 128:(kt + 1) * 128,
                                       ngi * 512:(ngi + 1) * 512])
                      for st in range(2):
                          nc.tensor.matmul(
                              pso[st][:],
                              oT[:, kt * ROWS + st * 128: kt * ROWS + (st + 1) * 128],
                              wsl[:], start=(ki == 0), stop=(ki == KT - 1))
                  for st in range(2):
                      osb = p_os.tile([128, 512], dt.float32, tag="os")
                      nc.scalar.copy(osb[:], pso[st][:])
                      nc.sync.dma_start(
                          out_d[st * 128:(st + 1) * 128,
                                ngi * 512:(ngi + 1) * 512], osb[:])

    nc.compile()
    return nc


_CACHE = {}


def _host_constants():
    inv = 10000.0 ** (-np.arange(0, D, 2, dtype=np.float64) / D)
    t = np.arange(S, dtype=np.float64)
    fr = np.outer(t, inv)                      # [S, 64]
    cos = np.cos(fr).T.astype(np.float32)      # [64, S]
    sin = np.sin(fr).T.astype(np.float32)
    cos2 = np.vstack([cos, cos])
    sin2 = np.vstack([-sin, sin])
    ident = np.eye(128, dtype=np.float32)
    a = np.arange(128)
    diagneg = np.where(a[None, :] <= a[:, None], 0.0, NEG).astype(np.float32)
    triup = (a[None, :] > a[:, None]).astype(np.float32)
    ones = np.ones((1, 128), dtype=np.float32)
    return cos2, sin2, ident, diagneg, triup, ones


def kernel(hidden_states, Wq, Wk, Wv, Wo, fe1_w, fe1_b, fe2_w, fe2_b,
           r1_w, r1_b, r2_w, r2_b, r3_w, r3_b, router_noise):
    if "nc" not in _CACHE:
        _CACHE["nc"] = build()
    nc = _CACHE["nc"]

    hs = np.ascontiguousarray(
        np.asarray(hidden_states, dtype=np.float32).reshape(S, HID))
    Wq = np.asarray(Wq, np.float32)
    Wk = np.asarray(Wk, np.float32)
    Wv = np.asarray(Wv, np.float32)
    Wo = np.ascontiguousarray(np.asarray(Wo, np.float32))
    cos2, sin2, ident, diagneg, triup, ones = _host_constants()

    def ktile_cols(w, ktiles, ntiles, nwidth):
        # [K, N] -> [128, ktiles*ntiles*nwidth] with (k, t) slab layout
        return np.ascontiguousarray(
              np.concatenate([w[k * 128:(k + 1) * 128, :] for k in range(ktiles)],
                             axis=1))

    fe1 = np.asarray(fe1_w, np.float32)                       # [128,1024]
    fe2 = ktile_cols(np.asarray(fe2_w, np.float32), 8, 2, 128)
    r1 = ktile_cols(np.asarray(r1_w, np.float32), 2, 4, 128)
    r2 = ktile_cols(np.asarray(r2_w, np.float32), 4, 1, 128)
    r3 = np.asarray(r3_w, np.float32)                         # [128,1]
    b1 = np.asarray(fe1_b, np.float32).reshape(8, 128).T.copy()
    b2 = np.asarray(fe2_b, np.float32).reshape(2, 128).T.copy()
    rb1 = np.asarray(r1_b, np.float32).reshape(4, 128).T.copy()
    rb2 = np.asarray(r2_b, np.float32).reshape(1, 128).T.copy()
    rb3 = np.asarray(r3_b, np.float32).reshape(1, 1)
    noise = np.asarray(router_noise, np.float32).reshape(1, 1)

    in_maps = []
    for c in range(NCORES):
        wqkv = np.ascontiguousarray(np.concatenate(
              [Wq[:, c * 512:(c + 1) * 512],
               Wk[:, c * 128:(c + 1) * 128],
               Wv[:, c * 128:(c + 1) * 128]], axis=1))
        in_maps.append(dict(
              hs=hs, wqkv=wqkv, wo=Wo, cos2=cos2, sin2=sin2, ident=ident,
              diagneg=diagneg, triup=triup, ones=ones, fe1=fe1, fe2=fe2,
              r1=r1, r2=r2, r3=r3, b1=b1, b2=b2, rb1=rb1, rb2=rb2, rb3=rb3,
              noise=noise, eps=np.full((1, 1), 1e-8, np.float32)))

    res = run_bass_kernel_spmd(nc, in_maps, list(range(NCORES)))
    out = np.concatenate([res.results[c]["out_rows"] for c in range(NCORES)],
                           axis=0)
    return out.reshape(1, S, HID).astype(np.float32)



# revision 3
# speedup vs baseline: 1.0703x; 1.0703x over previous
"""Trainium2 Bass kernel for nn_LlamaAttention_61899068670751.

Sparse (streaming-LLM) attention layer, tensor-parallel over heads across 8
NeuronCores; core c owns q-heads [4c..4c+3] and kv-head c (GQA group = 4).

Key design points vs the v1 baseline:
  - hs is transposed + quantized to fp8e4 on the host; QKV projections run as
    fp8 DoubleRow matmuls (2 k-tiles per instruction, 0.5 cycles/row).
  - attention scores are computed TRANSPOSED (stationary = k block, moving =
    qT) so exp() output lands directly in the [key, query] layout needed by
    the PV matmul -- no per-block PE transposes and no PSUM->SBUF p copies.
  - o is accumulated as o_strm (sink+window mask) and o_mid (causal minus
    strm); softmax denominators via ones-vector matmuls; per-query scaling is
    applied once to oT (128 x S) instead of to p (S x S).
  - the tiny router MLP runs per-core from a replicated head-averaged Wq
    (rope commutes with the head average), eliminating the AllReduce.
  - o exchanged with bf16 AllToAlls (split 3 ways to overlap attention and
    the output projection); out rows computed with bf16 matmuls.
"""
import numpy as np
import ml_dtypes
from contextlib import ExitStack

import concourse.bacc as bacc
import concourse.mybir as mybir
import concourse.tile as tile
from concourse.bass_utils import run_bass_kernel_spmd

dt = mybir.dt
AF = mybir.ActivationFunctionType
ALU = mybir.AluOpType
AX = mybir.AxisListType
PM = mybir.MatmulPerfMode
BF16 = ml_dtypes.bfloat16
FP8 = ml_dtypes.float8_e4m3fn

NCORES = 8
S, H, KV, D, HID = 2048, 32, 8, 128, 4096
SINK, WIN, POOL = 128, 1024, 100
HLOC = H // NCORES          # 4 q heads per core
NBLK = S // 128             # 16 key/query blocks
NCH = 4                     # query chunks of 512
CH = 512
KT = HID // 128             # 32 contraction tiles
KP = KT // 2                # 16 fp8 pair-tiles
SCALE = 1.0 / float(np.sqrt(D))
NEG = -1.0e30
ROWS = S // NCORES          # 256 output rows per core

S_HS = 16.0                 # hs fp8 scale
S_W = 2048.0                # qkv weight fp8 scale
DEQ = 1.0 / (S_HS * S_W)    # per-operand dequant


def build():
    nc = bacc.Bacc("TRN2", target_bir_lowering=False, debug=False,
                   num_devices=NCORES)

    def din(name, shape, d):
        return nc.dram_tensor(name, shape, d, kind="ExternalInput").ap()

    hsT8_d = din("hsT8", [KP, 128, 2, S], dt.float8e4)
    wqkv8_d = din("wqkv8", [KP, 128, 2, 768], dt.float8e4)
    wo_d = din("wo", [HID, HID], dt.bfloat16)
    cos2_d = din("cos2", [128, S], dt.bfloat16)      # rope cos, q/k dequant folded
    sin2_d = din("sin2", [128, S], dt.bfloat16)
    hsp_d = din("hsp", [KT, 128, 2 * POOL], dt.bfloat16)   # pool cols of hsT (x16)
    wqa_d = din("wqa", [KT, 128, 128], dt.bfloat16)        # head-avg Wq / 16
    cosp_d = din("cosp", [128, 2 * POOL], dt.bfloat16)     # plain rope at pool cols
    sinp_d = din("sinp", [128, 2 * POOL], dt.bfloat16)
    ident_d = din("ident", [128, 128], dt.bfloat16)
    diagnegT_d = din("diagnegT", [128, 128], dt.float32)
    trilow_d = din("trilow", [128, 128], dt.bfloat16)
    oneskey_d = din("oneskey", [128, 1], dt.bfloat16)
    fe1_d = din("fe1", [128, 1024], dt.float32)
    fe2_d = din("fe2", [128, 8 * 256], dt.float32)
    r1_d = din("r1", [128, 2 * 512], dt.float32)
    r2_d = din("r2", [128, 4 * 128], dt.float32)
    r3_d = din("r3", [128, 1], dt.float32)
    b1_d = din("b1", [128, 8], dt.float32)
    b2_d = din("b2", [128, 2], dt.float32)
    rb1_d = din("rb1", [128, 4], dt.float32)
    rb2_d = din("rb2", [128, 1], dt.float32)
    rb3_d = din("rb3", [1, 1], dt.float32)
    noise_d = din("noise", [1, 1], dt.float32)
    eps_d = din("eps", [1, 1], dt.float32)
    ones_r_d = din("ones_r", [1, 128], dt.float32)

    out_d = nc.dram_tensor("out_rows", [ROWS, HID], dt.float32,
                           kind="ExternalOutput").ap()

    with tile.TileContext(nc) as tc, ExitStack() as top:
        const = top.enter_context(tc.tile_pool(name="const", bufs=1))
        persist = top.enter_context(tc.tile_pool(name="persist", bufs=1))
        dram = top.enter_context(tc.tile_pool(name="dram", bufs=1, space="DRAM"))

        ident = const.tile([128, 128], dt.bfloat16)
        diagnegT = const.tile([128, 128], dt.float32)
        trilow = const.tile([128, 128], dt.bfloat16)
        oneskey = const.tile([128, 1], dt.bfloat16)
        ones_r = const.tile([1, 128], dt.float32)
        cos2 = const.tile([128, S], dt.bfloat16)
        sin2 = const.tile([128, S], dt.bfloat16)
        for t_, d_ in ((ident, ident_d), (diagnegT, diagnegT_d),
                       (trilow, trilow_d), (oneskey, oneskey_d),
                       (ones_r, ones_r_d), (cos2, cos2_d), (sin2, sin2_d)):
            nc.sync.dma_start(t_[:], d_[:])

        qT = [persist.tile([128, S], dt.bfloat16, name=f"qT{h}", tag=f"qT{h}")
              for h in range(HLOC)]
        kT = persist.tile([128, S], dt.bfloat16)
        vN = persist.tile([128, S], dt.bfloat16)    # v natural, 16 key blocks
        mixb = persist.tile([128, 1], dt.float32)   # z broadcast
        zbar = persist.tile([128, 1], dt.float32)   # 1-z
        negmix = persist.tile([128, 1], dt.float32)  # -z

        # a2a bounce: A = heads 0,1 all rows; B/C = heads 2,3 row halves
        a2a_inA = dram.tile([NCORES, 2, 128, ROWS], dt.bfloat16)
        a2a_outA = dram.tile([NCORES, 2, 128, ROWS], dt.bfloat16)
        a2a_inB = dram.tile([NCORES, 2, 128, ROWS // 2], dt.bfloat16)
        a2a_outB = dram.tile([NCORES, 2, 128, ROWS // 2], dt.bfloat16)
        a2a_inC = dram.tile([NCORES, 2, 128, ROWS // 2], dt.bfloat16)
        a2a_outC = dram.tile([NCORES, 2, 128, ROWS // 2], dt.bfloat16)

        # ---- phase 1: QKV fp8 DoubleRow + rope + router feature ----------
        with ExitStack() as ph1:
            p_w8 = ph1.enter_context(tc.tile_pool(name="w8", bufs=1))
            p_hs8 = ph1.enter_context(tc.tile_pool(name="hs8", bufs=4))
            p_rope = ph1.enter_context(tc.tile_pool(name="rope", bufs=2))
            p_rsb = ph1.enter_context(tc.tile_pool(name="rsb", bufs=1))
            ps_acc = ph1.enter_context(
                tc.tile_pool(name="ps_acc", bufs=1, space="PSUM"))
            ps_tr = ph1.enter_context(
                tc.tile_pool(name="ps_tr", bufs=1, space="PSUM"))
            ps_rt = ph1.enter_context(
                tc.tile_pool(name="ps_rt", bufs=1, space="PSUM"))

            # all qkv weights resident (24 KiB/partition)
            w8 = p_w8.tile([128, KP, 2, 768], dt.float8e4)
            for t in range(KP):
                nc.sync.dma_start(w8[:, t], wqkv8_d[t])

            # router feature inputs
            hsp = p_rsb.tile([128, KT, 2 * POOL], dt.bfloat16)
            wqa = p_rsb.tile([128, KT, 128], dt.bfloat16)
            cosp = p_rsb.tile([128, 2 * POOL], dt.bfloat16)
            sinp = p_rsb.tile([128, 2 * POOL], dt.bfloat16)
            nc.sync.dma_start(hsp[:], hsp_d.rearrange("k p f -> p k f"))
            nc.sync.dma_start(wqa[:], wqa_d.rearrange("k p f -> p k f"))
            nc.sync.dma_start(cosp[:], cosp_d[:])
            nc.sync.dma_start(sinp[:], sinp_d[:])

            rt_ps = ps_rt.tile([128, 2 * POOL], dt.float32)

            for g in range(NCH):
                s0 = g * CH
                accs = [ps_acc.tile([128, CH], dt.float32, tag=f"acc{i}",
                                    name=f"acc{i}") for i in range(6)]
                for t in range(KP):
                    hs8 = p_hs8.tile([128, 2, CH], dt.float8e4, tag="hs8")
                    nc.sync.dma_start(hs8[:], hsT8_d[t, :, :, s0:s0 + CH])
                    for i in range(6):
                        nc.tensor.matmul(
                            accs[i][:], w8[:, t, :, i * 128:(i + 1) * 128],
                            hs8[:], start=(t == 0), stop=(t == KP - 1),
                            perf_mode=PM.DoubleRow)
                if g == 0:
                    # router: q_avgT = sum_k wqa[k].T @ hsp[k]
                    for k in range(KT):
                        nc.tensor.matmul(rt_ps[:], wqa[:, k], hsp[:, k],
                                         start=(k == 0), stop=(k == KT - 1))

                # rope for q heads (0..3) and k (4); cos2/sin2 carry dequant
                for i in range(5):
                    dest = qT[i] if i < HLOC else kT
                    lin = p_rope.tile([128, CH], dt.bfloat16, tag="lin")
                    rot = p_rope.tile([128, CH], dt.bfloat16, tag="rot")
                    t1 = p_rope.tile([128, CH], dt.bfloat16, tag="t1")
                    nc.scalar.copy(lin[:], accs[i][:])
                    nc.sync.dma_start(rot[0:64, :], lin[64:128, :])
                    nc.sync.dma_start(rot[64:128, :], lin[0:64, :])
                    nc.vector.tensor_tensor(t1[:], lin[:], cos2[:, s0:s0 + CH],
                                            ALU.mult)
                    nc.vector.tensor_tensor(rot[:], rot[:], sin2[:, s0:s0 + CH],
                                            ALU.mult)
                    nc.vector.tensor_tensor(dest[:, s0:s0 + CH], t1[:], rot[:],
                                            ALU.add)
                # v: dequant copy then transpose to natural layout
                vT = p_rope.tile([128, CH], dt.bfloat16, tag="vT")
                nc.scalar.activation(vT[:], accs[5][:], AF.Copy, scale=DEQ)
                ptr = ps_tr.tile([128, CH], dt.bfloat16, tag="tr")
                for ss in range(4):
                    nc.tensor.transpose(ptr[:, ss * 128:(ss + 1) * 128],
                                        vT[:, ss * 128:(ss + 1) * 128],
                                        ident[:])
                nc.vector.tensor_copy(vN[:, s0:s0 + CH], ptr[:])

            # router rope + feature (q_avg is true-scaled: hsp x16, wqa /16)
            rlin = p_rsb.tile([128, 2 * POOL], dt.bfloat16)
            rrot = p_rsb.tile([128, 2 * POOL], dt.bfloat16)
            rt1 = p_rsb.tile([128, 2 * POOL], dt.bfloat16)
            nc.scalar.copy(rlin[:], rt_ps[:])
            nc.sync.dma_start(rrot[0:64, :], rlin[64:128, :])
            nc.sync.dma_start(rrot[64:128, :], rlin[0:64, :])
            nc.vector.tensor_tensor(rt1[:], rlin[:], cosp[:], ALU.mult)
            nc.vector.tensor_tensor(rrot[:], rrot[:], sinp[:], ALU.mult)
            nc.vector.tensor_tensor(rt1[:], rt1[:], rrot[:], ALU.add)
            feat = p_rsb.tile([128, 1], dt.float32)
            nc.vector.tensor_reduce(feat[:], rt1[:], AX.X, ALU.add)
            featg = persist.tile([128, 1], dt.float32)
            nc.scalar.activation(featg[:], feat[:], AF.Copy,
                                 scale=1.0 / (2 * POOL))

        # ---- router MLP (tiny, replicated) -------------------------------
        with ExitStack() as phm:
            p_mlp = phm.enter_context(tc.tile_pool(name="mlp", bufs=1))
            ps_m = phm.enter_context(
                tc.tile_pool(name="ps_m", bufs=1, space="PSUM"))

            fe1 = p_mlp.tile([128, 1024], dt.float32)
            fe2 = p_mlp.tile([128, 8 * 256], dt.float32)
            r1w = p_mlp.tile([128, 2 * 512], dt.float32)
            r2w = p_mlp.tile([128, 4 * 128], dt.float32)
            r3w = p_mlp.tile([128, 1], dt.float32)
            b1 = p_mlp.tile([128, 8], dt.float32)
            b2 = p_mlp.tile([128, 2], dt.float32)
            rb1 = p_mlp.tile([128, 4], dt.float32)
            rb2 = p_mlp.tile([128, 1], dt.float32)
            rb3 = p_mlp.tile([1, 1], dt.float32)
            noise = p_mlp.tile([1, 1], dt.float32)
            epsb = p_mlp.tile([1, 1], dt.float32)
            for t_, d_ in ((fe1, fe1_d), (fe2, fe2_d), (r1w, r1_d),
                           (r2w, r2_d), (r3w, r3_d), (b1, b1_d), (b2, b2_d),
                           (rb1, rb1_d), (rb2, rb2_d), (rb3, rb3_d),
                           (noise, noise_d), (epsb, eps_d)):
                nc.sync.dma_start(t_[:], d_[:])

            mlp_tmp = []

            def mlp_layer(vec_in, w_sb, ktiles, ntiles, bias, act, nwidth=128):
                out_r = p_mlp.tile([128, max(ntiles, 1)], dt.float32,
                                   name=f"mlpv{len(mlp_tmp)}")
                mlp_tmp.append(out_r)
                ps = ps_m.tile([128, max(ntiles, 1)], dt.float32, tag="mlp",
                               name="mlpps")
                for t in range(ntiles):
                    for k in range(ktiles):
                        nc.tensor.matmul(
                            ps[:, t:t + 1],
                            w_sb[:, (k * ntiles + t) * nwidth:
                                 (k * ntiles + t) * nwidth + nwidth],
                            vec_in[:, k:k + 1],
                            start=(k == 0), stop=(k == ktiles - 1))
                for t in range(ntiles):
                    nc.scalar.activation(out_r[:, t:t + 1], ps[:, t:t + 1],
                                         act, bias=bias[:, t:t + 1])
                return out_r

            h1 = mlp_layer(featg, fe1, 1, 8, b1, AF.Silu)
            h2 = mlp_layer(h1, fe2, 8, 2, b2, AF.Identity)
            h3 = mlp_layer(h2, r1w, 2, 4, rb1, AF.Silu)
            h4 = mlp_layer(h3, r2w, 4, 1, rb2, AF.Silu)
            lps = ps_m.tile([1, 1], dt.float32, tag="mlp")
            nc.tensor.matmul(lps[:], r3w[:], h4[:], start=True, stop=True)
            logits = p_mlp.tile([1, 1], dt.float32)
            nc.scalar.activation(logits[:], lps[:], AF.Identity, bias=rb3[:])
            l1 = p_mlp.tile([1, 1], dt.float32)
            l2 = p_mlp.tile([1, 1], dt.float32)
            nc.scalar.activation(l1[:], noise[:], AF.Ln, bias=epsb[:])
            nc.scalar.activation(l2[:], l1[:], AF.Ln, bias=epsb[:], scale=-1.0)
            zin = p_mlp.tile([1, 1], dt.float32)
            nc.vector.tensor_tensor(zin[:], logits[:], l2[:], ALU.subtract)
            zsoft = p_mlp.tile([1, 1], dt.float32)
            nc.scalar.activation(zsoft[:], zin[:], AF.Sigmoid)
            zhard = p_mlp.tile([1, 1], dt.float32)
            nc.vector.tensor_scalar(zhard[:], zsoft[:], 0.5, None, ALU.is_gt)
            mps = ps_m.tile([128, 1], dt.float32, tag="mlp")
            nc.tensor.matmul(mps[:], ones_r[:], zhard[:], start=True, stop=True)
            nc.scalar.copy(mixb[:], mps[:])
            nc.vector.tensor_scalar(zbar[:], mixb[:], -1.0, 1.0, ALU.mult,
                                    ALU.add)
            nc.vector.tensor_scalar(negmix[:], mixb[:], -1.0, None, ALU.mult)

        # ---- phase 2: attention ------------------------------------------
        with ExitStack() as ph2:
            p_e = ph2.enter_context(tc.tile_pool(name="eband", bufs=2))
            p_tri = ph2.enter_context(tc.tile_pool(name="tri", bufs=2))
            p_cb = ph2.enter_context(tc.tile_pool(name="cmb", bufs=2))
            ps_sc = ph2.enter_context(
                tc.tile_pool(name="ps_sc", bufs=2, space="PSUM"))
            ps_os = ph2.enter_context(
                tc.tile_pool(name="ps_os", bufs=1, space="PSUM"))
            ps_om = ph2.enter_context(
                tc.tile_pool(name="ps_om", bufs=1, space="PSUM"))
            ps_sm = ph2.enter_context(
                tc.tile_pool(name="ps_sm", bufs=1, space="PSUM"))

            def acc_matmuls(dst_tile, ops, stationary):
                """Emit an accumulation group; ops = (J, lo, hi, src_ap).
                start=True on the first op touching each 128-col block."""
                written = set()
                for n, (J, lo, hi, src) in enumerate(ops):
                    blocks = set(range(lo // 128, hi // 128))
                    fresh = not (blocks & written)
                    assert fresh or blocks <= written, (c, n, ops)
                    written |= blocks
                    nc.tensor.matmul(
                        dst_tile[:, lo:hi] if dst_tile.shape[0] > 1
                        else dst_tile[0:1, lo:hi],
                        stationary(J), src,
                        start=fresh, stop=(n == len(ops) - 1),
                        skip_group_check=True)

            for h in range(HLOC):
                for c in range(NCH):
                    q0 = c * CH
                    nJ = 4 * c + 4          # key blocks 0..nJ-1
                    eT = p_e.tile([128, NBLK, CH], dt.bfloat16, tag="eT")
                    # masked copies for J = I-8 (I in chunk): 4 slots
                    etri = p_tri.tile([128, 2, 4, 128], dt.bfloat16,
                                      tag="etri")  # [mid|strm, slot]

                    # scores (transposed) + exp, two J blocks per psum tile
                    for J0 in range(0, nJ, 2):
                        sc = ps_sc.tile([128, 1024], dt.float32, tag="sc")
                        ws = []
                        for jj in range(2):
                            J = J0 + jj
                            lo = max(q0, J * 128)
                            w = (c + 1) * CH - lo
                            ws.append(w)
                            nc.tensor.matmul(
                                sc[:, jj * CH: jj * CH + w],
                                kT[:, J * 128:(J + 1) * 128],
                                qT[h][:, lo:lo + w],
                                start=True, stop=True)
                            if J >= 4 * c:  # diag block: causal mask
                                nc.vector.tensor_tensor(
                                    sc[:, jj * CH: jj * CH + 128],
                                    sc[:, jj * CH: jj * CH + 128],
                                    diagnegT[:], ALU.add)
                        if ws[0] == CH and ws[1] == CH:
                            nc.scalar.activation(
                                eT[:, J0:J0 + 2, :].rearrange(
                                    "p a b -> p (a b)"),
                                sc[:], AF.Exp, scale=SCALE)
                        else:
                            for jj in range(2):
                                J = J0 + jj
                                lo = max(q0, J * 128) - q0
                                nc.scalar.activation(
                                    eT[:, J, lo:CH],
                                    sc[:, jj * CH: jj * CH + ws[jj]],
                                    AF.Exp, scale=SCALE)

                    # triangle masks at J = I-8 for I in chunk (J>=1)
                    tslot = {}
                    for ii in range(4):
                        I = 4 * c + ii
                        J = I - 8
                        if J < 1:
                            continue
                        tslot[J] = ii
                        icol = I * 128 - q0
                        nc.vector.tensor_tensor(
                            etri[:, 0, ii, :], eT[:, J, icol:icol + 128],
                            trilow[:], ALU.mult)
                        nc.vector.tensor_tensor(
                            etri[:, 1, ii, :], eT[:, J, icol:icol + 128],
                            etri[:, 0, ii, :], ALU.subtract)

                    # op lists -------------------------------------------------
                    full_ops = []
                    for J in range(nJ):
                        lo = max(q0, J * 128) - q0
                        full_ops.append((J, lo, CH, eT[:, J, lo:CH]))
                    mid_ops = []
                    for J in range(1, nJ):
                        ilo = max(4 * c, J + 9)
                        if ilo <= 4 * c + 3:
                            lo = ilo * 128 - q0
                            mid_ops.append((J, lo, CH, eT[:, J, lo:CH]))
                        if J in tslot:
                            t = tslot[J]
                            mid_ops.append(
                                (J, t * 128, t * 128 + 128, etri[:, 0, t, :]))
                    strm_ops = [(0, 0, CH, eT[:, 0, 0:CH])]   # sink
                    for J in range(max(1, 4 * c - 7), nJ):
                        lo = max(q0, J * 128) - q0
                        hi = min(CH, (J + 8) * 128 - q0)
                        strm_ops.append((J, lo, hi, eT[:, J, lo:hi]))
                        if J in tslot:
                            t = tslot[J]
                            strm_ops.append(
                                (J, t * 128, t * 128 + 128, etri[:, 1, t, :]))

                    # denominators (ones-vector matmuls) and PV accumulations
                    sums_f = ps_sm.tile([1, CH], dt.float32, tag="sf")
                    acc_matmuls(sums_f, full_ops, lambda J: oneskey[:])
                    o_s = ps_os.tile([128, CH], dt.float32, tag="os")
                    acc_matmuls(o_s, strm_ops,
                                lambda J: vN[:, J * 128:(J + 1) * 128])
                    if mid_ops:
                        sums_m = ps_sm.tile([1, CH], dt.float32, tag="sm")
                        acc_matmuls(sums_m, mid_ops, lambda J: oneskey[:])
                        o_m = ps_om.tile([128, CH], dt.float32, tag="om")
                        acc_matmuls(o_m, mid_ops,
                                    lambda J: vN[:, J * 128:(J + 1) * 128])

                    # combine + scale
                    scmb = p_cb.tile([1, CH], dt.float32, tag="scmb")
                    ocmb = p_cb.tile([128, CH], dt.float32, tag="ocmb")
                    if mid_ops:
                        # covered mid cols: [mlo, CH); others: strm == full
                        mlo = min(lo for _, lo, _, _ in mid_ops)
                        tmp = p_cb.tile([1, CH], dt.float32, tag="stmp")
                        nc.vector.tensor_scalar(
                            tmp[0:1, mlo:CH], sums_m[0:1, mlo:CH],
                            negmix[0:1, 0:1], None, ALU.mult)
                        if mlo > 0:
                            nc.vector.memset(tmp[0:1, 0:mlo], 0.0)
                        nc.vector.tensor_tensor(scmb[:], tmp[:], sums_f[:],
                                                ALU.add)
                        otmp = p_cb.tile([128, CH], dt.float32, tag="otmp")
                        nc.vector.tensor_scalar(
                            otmp[:, mlo:CH], o_m[:, mlo:CH], zbar[:, 0:1],
                            None, ALU.mult)
                        if mlo > 0:
                            nc.vector.memset(otmp[:, 0:mlo], 0.0)
                        nc.vector.tensor_tensor(ocmb[:], otmp[:], o_s[:],
                                                ALU.add)
                    else:
                        nc.vector.tensor_copy(scmb[:], sums_f[:])
                        nc.vector.tensor_copy(ocmb[:], o_s[:])
                    recip = p_cb.tile([1, CH], dt.float32, tag="recip")
                    nc.vector.reciprocal(recip[:], scmb[:])
                    rbc = p_cb.tile([128, CH], dt.float32, tag="rbc")
                    nc.gpsimd.partition_broadcast(rbc[:], recip[:])
                    osb = p_cb.tile([128, CH], dt.bfloat16, tag="osb")
                    nc.vector.tensor_tensor(osb[:], ocmb[:], rbc[:], ALU.mult)

                    # scatter to a2a bounce buffers
                    if h < 2:
                        nc.sync.dma_start(a2a_inA[2 * c, h, :, :],
                                          osb[:, 0:ROWS])
                        nc.sync.dma_start(a2a_inA[2 * c + 1, h, :, :],
                                          osb[:, ROWS:CH])
                    else:
                        hh = h - 2
                        for q in range(4):
                            dst = a2a_inB if q % 2 == 0 else a2a_inC
                            nc.sync.dma_start(
                                dst[4 * c // 2 + q // 2, hh, :, :],
                                osb[:, q * 128:(q + 1) * 128])
                if h == 1:
                    nc.gpsimd.collective_compute(
                        "AllToAll", ALU.bypass,
                        replica_groups=[list(range(NCORES))],
                        ins=[a2a_inA.opt()], outs=[a2a_outA.opt()])
                if h == 3:
                    nc.gpsimd.collective_compute(
                        "AllToAll", ALU.bypass,
                        replica_groups=[list(range(NCORES))],
                        ins=[a2a_inB.opt()], outs=[a2a_outB.opt()])
            nc.gpsimd.collective_compute(
                "AllToAll", ALU.bypass,
                replica_groups=[list(range(NCORES))],
                ins=[a2a_inC.opt()], outs=[a2a_outC.opt()])

        # ---- phase 3: output projection ----------------------------------
        with ExitStack() as ph3:
            p_oT = ph3.enter_context(tc.tile_pool(name="oT", bufs=1))
            p_wo = ph3.enter_context(tc.tile_pool(name="wo", bufs=24))
            p_os3 = ph3.enter_context(tc.tile_pool(name="outsb", bufs=2))
            ps_w = ph3.enter_context(
                tc.tile_pool(name="ps_w", bufs=4, space="PSUM"))

            # oT[k-tile g] = head (p, t) block; rows split by a2a phase
            oT = p_oT.tile([128, KT, ROWS], dt.bfloat16)
            for p in range(NCORES):
                for t in range(HLOC):
                    g = p * HLOC + t
                    if t < 2:
                        nc.sync.dma_start(oT[:, g, :], a2a_outA[p, t, :, :])
                    else:
                        nc.sync.dma_start(oT[:, g, 0:128],
                                          a2a_outB[p, t - 2, :, :])
                        nc.sync.dma_start(oT[:, g, 128:256],
                                          a2a_outC[p, t - 2, :, :])

            # row-half 0 first (B lands before C)
            for st in range(2):
                for ngi in range(8):
                    pso = ps_w.tile([128, CH], dt.float32, tag="wo")
                    for g in range(KT):
                        wsl = p_wo.tile([128, CH], dt.bfloat16, tag="wo")
                        nc.sync.dma_start(
                            wsl[:], wo_d[g * 128:(g + 1) * 128,
                                         ngi * CH:(ngi + 1) * CH])
                        nc.tensor.matmul(
                            pso[:], oT[:, g, st * 128:(st + 1) * 128],
                            wsl[:], start=(g == 0), stop=(g == KT - 1))
                    osb = p_os3.tile([128, CH], dt.float32, tag="os")
                    nc.scalar.copy(osb[:], pso[:])
                    nc.sync.dma_start(
                        out_d[st * 128:(st + 1) * 128,
                              ngi * CH:(ngi + 1) * CH], osb[:])

    nc.compile()
    return nc


_CACHE = {}


def _host_constants():
    inv = 10000.0 ** (-np.arange(0, D, 2, dtype=np.float64) / D)
    t = np.arange(S, dtype=np.float64)
    fr = np.outer(t, inv)                      # [S, 64]
    cos = np.cos(fr).T.astype(np.float64)      # [64, S]
    sin = np.sin(fr).T.astype(np.float64)
    cos2 = np.vstack([cos, cos])
    sin2 = np.vstack([-sin, sin])
    a = np.arange(128)
    ident = np.eye(128, dtype=np.float32)
    diagnegT = np.where(a[:, None] <= a[None, :], 0.0, NEG).astype(np.float32)
    trilow = (a[:, None] <= a[None, :]).astype(np.float32)
    return cos2, sin2, ident, diagnegT, trilow


def kernel(hidden_states, Wq, Wk, Wv, Wo, fe1_w, fe1_b, fe2_w, fe2_b,
           r1_w, r1_b, r2_w, r2_b, r3_w, r3_b, router_noise):
    if "nc" not in _CACHE:
        _CACHE["nc"] = build()
    nc = _CACHE["nc"]

    hs = np.asarray(hidden_states, np.float32).reshape(S, HID)
    Wq = np.asarray(Wq, np.float32)
    Wk = np.asarray(Wk, np.float32)
    Wv = np.asarray(Wv, np.float32)
    Wo = np.asarray(Wo, np.float32)

    cos2, sin2, ident, diagnegT, trilow = _host_constants()

    # hs transposed, scaled, fp8-quantized, pair-tile layout
    hsT = np.ascontiguousarray(hs.T) * S_HS
    hsT8 = np.clip(hsT, -448, 448).astype(FP8)
    hsT8 = hsT8.reshape(KP, 2, 128, S).transpose(0, 2, 1, 3).copy()

    pool_idx = np.r_[0:POOL, S - POOL:S]
    hsp = (hs.T[:, pool_idx] * S_HS).astype(BF16).reshape(KT, 128, 2 * POOL)
    wqa = (Wq.reshape(HID, H, D).mean(axis=1) / S_HS).astype(
        BF16).reshape(KT, 128, 128)
    cosp = np.vstack([np.ascontiguousarray(cos2[:, pool_idx])])
    sinp = np.vstack([np.ascontiguousarray(sin2[:, pool_idx])])
    # main cos/sin carry the q/k dequant
    cos2m = (cos2 * DEQ).astype(BF16)
    sin2m = (sin2 * DEQ).astype(BF16)

    def ktile_cols(w, ktiles):
        return np.ascontiguousarray(
            np.concatenate([w[k * 128:(k + 1) * 128, :] for k in range(ktiles)],
                           axis=1))

    fe1 = np.asarray(fe1_w, np.float32)
    fe2 = ktile_cols(np.asarray(fe2_w, np.float32), 8)
    r1 = ktile_cols(np.asarray(r1_w, np.float32), 2)
    r2 = ktile_cols(np.asarray(r2_w, np.float32), 4)
    r3 = np.asarray(r3_w, np.float32)
    b1 = np.asarray(fe1_b, np.float32).reshape(8, 128).T.copy()
    b2 = np.asarray(fe2_b, np.float32).reshape(2, 128).T.copy()
    rb1 = np.asarray(r1_b, np.float32).reshape(4, 128).T.copy()
    rb2 = np.asarray(r2_b, np.float32).reshape(1, 128).T.copy()
    rb3 = np.asarray(r3_b, np.float32).reshape(1, 1)
    noise = np.asarray(router_noise, np.float32).reshape(1, 1)
    wo_bf = np.ascontiguousarray(Wo.astype(BF16))

    in_maps = []
    for c in range(NCORES):
        wqkv = np.concatenate(
            [Wq[:, c * 512:(c + 1) * 512],
             Wk[:, c * 128:(c + 1) * 128],
             Wv[:, c * 128:(c + 1) * 128]], axis=1) * S_W
        wqkv8 = np.clip(wqkv, -448, 448).astype(FP8)
        wqkv8 = wqkv8.reshape(KP, 2, 128, 768).transpose(0, 2, 1, 3).copy()
        in_maps.append(dict(
            hsT8=hsT8.view(np.uint8), wqkv8=wqkv8.view(np.uint8),
            wo=wo_bf, cos2=cos2m, sin2=sin2m,
            hsp=hsp, wqa=wqa,
            cosp=cosp.astype(BF16), sinp=sinp.astype(BF16),
            ident=ident.astype(BF16), diagnegT=diagnegT,
            trilow=trilow.astype(BF16),
            oneskey=np.ones((128, 1), BF16),
            fe1=fe1, fe2=fe2, r1=r1, r2=r2, r3=r3, b1=b1, b2=b2,
            rb1=rb1, rb2=rb2, rb3=rb3, noise=noise,
            eps=np.full((1, 1), 1e-8, np.float32),
            ones_r=np.ones((1, 128), np.float32)))

    res = run_bass_kernel_spmd(nc, in_maps, list(range(NCORES)))
    out = np.concatenate([res.results[c]["out_rows"] for c in range(NCORES)],
                         axis=0)
    return out.reshape(1, S, HID).astype(np.float32)


# revision 8
# speedup vs baseline: 1.8529x; 1.7313x over previous
"""Trainium2 Bass kernel for nn_LlamaAttention_61899068670751.

Sparse (streaming-LLM) attention layer, tensor-parallel over heads across 8
NeuronCores; core c owns q-heads [4c..4c+3] and kv-head c (GQA group = 4).

Key design points vs the v1 baseline:
  - hs is transposed + quantized to fp8e4 on the host; QKV projections run as
    fp8 DoubleRow matmuls (2 k-tiles per instruction, 0.5 cycles/row).
  - attention scores are computed TRANSPOSED (stationary = k block, moving =
    qT) so exp() output lands directly in the [key, query] layout needed by
    the PV matmul -- no per-block PE transposes and no PSUM->SBUF p copies.
  - o is accumulated as o_strm (sink+window mask) and o_mid (causal minus
    strm); softmax denominators via ones-vector matmuls; per-query scaling is
    applied once to oT (128 x S) instead of to p (S x S).
  - the tiny router MLP runs per-core from a replicated head-averaged Wq
    (rope commutes with the head average), eliminating the AllReduce.
  - o exchanged with two bf16 AllToAlls; output projection in bf16 with the
    contraction ordered so peers' heads 0-1 (first AllToAll) are consumed
    while the second AllToAll is still in flight.
  - DMas are batched aggressively (whole-chunk transfers, packed constant
    blobs) -- the HWDGE fixed cost (~625 ns per dma_start) dominates
    otherwise.
"""
import numpy as np
import ml_dtypes
from contextlib import ExitStack

import concourse.bacc as bacc
import concourse.mybir as mybir
import concourse.tile as tile
from concourse.bass_utils import run_bass_kernel_spmd

dt = mybir.dt
AF = mybir.ActivationFunctionType
ALU = mybir.AluOpType
AX = mybir.AxisListType
PM = mybir.MatmulPerfMode
BF16 = ml_dtypes.bfloat16
FP8 = ml_dtypes.float8_e4m3fn

NCORES = 8
S, H, KV, D, HID = 2048, 32, 8, 128, 4096
SINK, WIN, POOL = 128, 1024, 100
HLOC = H // NCORES          # 4 q heads per core
NBLK = S // 128             # 16 key/query blocks
NCH = 4                     # query chunks of 512
CH = 512
KT = HID // 128             # 32 contraction tiles
KP = KT // 2                # 16 fp8 pair-tiles
SCALE = 1.0 / float(np.sqrt(D))
NEG = -1.0e30
ROWS = S // NCORES          # 256 output rows per core

S_HS = 16.0                 # hs fp8 scale
S_W = 2048.0                # qkv weight fp8 scale
DEQ = 1.0 / (S_HS * S_W)    # per-operand dequant

# packed bf16 const blob column offsets
_B_IDENT = 0
_B_TRIL = 128
_B_ONES = 256
_B_COS = 257
_B_SIN = _B_COS + S
_B_COSP = _B_SIN + S
_B_SINP = _B_COSP + 2 * POOL
_B_WQA = _B_SINP + 2 * POOL
_B_HSP = _B_WQA + KT * 128
_B_END = _B_HSP + KT * 2 * POOL
# packed fp32 blob: diagnegT | mlp weights
_F_DIAG = 0
_F_FE1 = 128
_F_FE2 = _F_FE1 + 1024
_F_R1 = _F_FE2 + 2048
_F_R2 = _F_R1 + 1024
_F_R3 = _F_R2 + 512
_F_B1 = _F_R3 + 1
_F_B2 = _F_B1 + 8
_F_RB1 = _F_B2 + 2
_F_RB2 = _F_RB1 + 4
_F_MISC = _F_RB2 + 1        # [rb3, noise, eps] on partition 0
_F_ONESR = _F_MISC + 3      # [1, 128] ones row on partition 0
_F_END = _F_ONESR + 128


def build():
    nc = bacc.Bacc("TRN2", target_bir_lowering=False, debug=False,
                   num_devices=NCORES)

    def din(name, shape, d):
        return nc.dram_tensor(name, shape, d, kind="ExternalInput").ap()

    hsT8_d = din("hsT8", [128, KT, S], dt.float8e4)
    wqkv8_d = din("wqkv8", [128, KT, 768], dt.float8e4)
    wo_d = din("wo", [HID, HID], dt.bfloat16)
    blob_d = din("blob", [128, _B_END], dt.bfloat16)
    fblob_d = din("fblob", [128, _F_END], dt.float32)

    out_d = nc.dram_tensor("out_rows", [ROWS, HID], dt.float32,
                           kind="ExternalOutput").ap()

    with tile.TileContext(nc) as tc, ExitStack() as top:
        const = top.enter_context(tc.tile_pool(name="const", bufs=1))
        persist = top.enter_context(tc.tile_pool(name="persist", bufs=1))
        dram = top.enter_context(tc.tile_pool(name="dram", bufs=1, space="DRAM"))

        blob = const.tile([128, _B_END], dt.bfloat16)
        fblob = const.tile([128, _F_END], dt.float32)
        nc.sync.dma_start(blob[:], blob_d[:])
        nc.sync.dma_start(fblob[:], fblob_d[:])
        ident = blob[:, _B_IDENT:_B_IDENT + 128]
        trilow = blob[:, _B_TRIL:_B_TRIL + 128]
        oneskey = blob[:, _B_ONES:_B_ONES + 1]
        cos2 = blob[:, _B_COS:_B_COS + S]
        sin2 = blob[:, _B_SIN:_B_SIN + S]
        cosp = blob[:, _B_COSP:_B_COSP + 2 * POOL]
        sinp = blob[:, _B_SINP:_B_SINP + 2 * POOL]
        wqa = blob[:, _B_WQA:_B_WQA + KT * 128].rearrange(
            "p (k f) -> p k f", f=128)
        hsp = blob[:, _B_HSP:_B_HSP + KT * 2 * POOL].rearrange(
            "p (k f) -> p k f", f=2 * POOL)
        diagnegT = fblob[:, _F_DIAG:_F_DIAG + 128]

        qT = [persist.tile([128, S], dt.bfloat16, name=f"qT{h}", tag=f"qT{h}")
              for h in range(HLOC)]
        kT = persist.tile([128, S], dt.bfloat16)
        vN = persist.tile([128, S], dt.bfloat16)    # v natural, 16 key blocks
        mixb = persist.tile([128, 1], dt.float32)   # z broadcast
        zbar = persist.tile([128, 1], dt.float32)   # 1-z
        negmix = persist.tile([128, 1], dt.float32)  # -z

        # a2a bounce: A = heads 0,1; B = heads 2,3
        a2a_inA = dram.tile([NCORES, 2, 128, ROWS], dt.bfloat16)
        a2a_outA = dram.tile([NCORES, 2, 128, ROWS], dt.bfloat16)
        a2a_inB = dram.tile([NCORES, 2, 128, ROWS], dt.bfloat16)
        a2a_outB = dram.tile([NCORES, 2, 128, ROWS], dt.bfloat16)

        # ---- phase 1: QKV fp8 DoubleRow + rope + router feature ----------
        with ExitStack() as ph1:
            p_w8 = ph1.enter_context(tc.tile_pool(name="w8", bufs=1))
            p_hs8 = ph1.enter_context(tc.tile_pool(name="hs8", bufs=2))
            p_rope = ph1.enter_context(tc.tile_pool(name="rope", bufs=2))
            p_rsb = ph1.enter_context(tc.tile_pool(name="rsb", bufs=1))
            ps_acc = ph1.enter_context(
                tc.tile_pool(name="ps_acc", bufs=1, space="PSUM"))
            ps_tr = ph1.enter_context(
                tc.tile_pool(name="ps_tr", bufs=1, space="PSUM"))
            ps_rt = ph1.enter_context(
                tc.tile_pool(name="ps_rt", bufs=1, space="PSUM"))

            # all qkv weights resident (24 KiB/partition), one DMA
            w8 = p_w8.tile([128, KT, 768], dt.float8e4)
            nc.sync.dma_start(w8[:], wqkv8_d[:])

            rt_ps = ps_rt.tile([128, 2 * POOL], dt.float32)

            for g in range(NCH):
                s0 = g * CH
                accs = [ps_acc.tile([128, CH], dt.float32, tag=f"acc{i}",
                                    name=f"acc{i}") for i in range(6)]
                hs8 = p_hs8.tile([128, KT, CH], dt.float8e4, tag="hs8")
                nc.sync.dma_start(hs8[:], hsT8_d[:, :, s0:s0 + CH])
                for t in range(KP):
                    for i in range(6):
                        nc.tensor.matmul(
                            accs[i][:],
                            w8[:, 2 * t:2 * t + 2, i * 128:(i + 1) * 128],
                            hs8[:, 2 * t:2 * t + 2, :],
                            start=(t == 0), stop=(t == KP - 1),
                            perf_mode=PM.DoubleRow)
                if g == 0:
                    # router: q_avgT = sum_k wqa[k].T @ hsp[k]
                    for k in range(KT):
                        nc.tensor.matmul(rt_ps[:], wqa[:, k], hsp[:, k],
                                         start=(k == 0), stop=(k == KT - 1))

                # rope for q heads (0..3) and k (4); cos2/sin2 carry dequant
                lin = p_rope.tile([128, 5, CH], dt.bfloat16, tag="lin")
                rot = p_rope.tile([128, 5, CH], dt.bfloat16, tag="rot")
                for i in range(5):
                    nc.scalar.copy(lin[:, i], accs[i][:])
                lin2 = lin.rearrange("p a b -> p (a b)")
                rot2 = rot.rearrange("p a b -> p (a b)")
                nc.sync.dma_start(rot2[0:64, :], lin2[64:128, :])
                nc.sync.dma_start(rot2[64:128, :], lin2[0:64, :])
                for i in range(5):
                    dest = qT[i] if i < HLOC else kT
                    t1 = p_rope.tile([128, CH], dt.bfloat16, tag="t1")
                    nc.vector.tensor_tensor(t1[:], lin[:, i],
                                            cos2[:, s0:s0 + CH], ALU.mult)
                    nc.vector.tensor_tensor(rot[:, i], rot[:, i],
                                            sin2[:, s0:s0 + CH], ALU.mult)
                    nc.vector.tensor_tensor(dest[:, s0:s0 + CH], t1[:],
                                            rot[:, i], ALU.add)
                # v: dequant copy then transpose to natural layout
                vT = p_rope.tile([128, CH], dt.bfloat16, tag="vT")
                nc.scalar.activation(vT[:], accs[5][:], AF.Copy, scale=DEQ)
                ptr = ps_tr.tile([128, CH], dt.bfloat16, tag="tr")
                for ss in range(4):
                    nc.tensor.transpose(ptr[:, ss * 128:(ss + 1) * 128],
                                        vT[:, ss * 128:(ss + 1) * 128],
                                        ident[:])
                nc.vector.tensor_copy(vN[:, s0:s0 + CH], ptr[:])

            # router rope + feature (q_avg is true-scaled: hsp x16, wqa /16)
            rlin = p_rsb.tile([128, 2 * POOL], dt.bfloat16)
            rrot = p_rsb.tile([128, 2 * POOL], dt.bfloat16)
            rt1 = p_rsb.tile([128, 2 * POOL], dt.bfloat16)
            nc.scalar.copy(rlin[:], rt_ps[:])
            nc.sync.dma_start(rrot[0:64, :], rlin[64:128, :])
            nc.sync.dma_start(rrot[64:128, :], rlin[0:64, :])
            nc.vector.tensor_tensor(rt1[:], rlin[:], cosp[:], ALU.mult)
            nc.vector.tensor_tensor(rrot[:], rrot[:], sinp[:], ALU.mult)
            nc.vector.tensor_tensor(rt1[:], rt1[:], rrot[:], ALU.add)
            feat = p_rsb.tile([128, 1], dt.float32)
            nc.vector.tensor_reduce(feat[:], rt1[:], AX.X, ALU.add)
            featg = persist.tile([128, 1], dt.float32)
            nc.scalar.activation(featg[:], feat[:], AF.Copy,
                                 scale=1.0 / (2 * POOL))

        # ---- router MLP (tiny, replicated) -------------------------------
        with ExitStack() as phm:
            p_mlp = phm.enter_context(tc.tile_pool(name="mlp", bufs=1))
            ps_m = phm.enter_context(
                tc.tile_pool(name="ps_m", bufs=1, space="PSUM"))

            fe1 = fblob[:, _F_FE1:_F_FE1 + 1024]
            fe2 = fblob[:, _F_FE2:_F_FE2 + 2048]
            r1w = fblob[:, _F_R1:_F_R1 + 1024]
            r2w = fblob[:, _F_R2:_F_R2 + 512]
            r3w = fblob[:, _F_R3:_F_R3 + 1]
            b1 = fblob[:, _F_B1:_F_B1 + 8]
            b2 = fblob[:, _F_B2:_F_B2 + 2]
            rb1 = fblob[:, _F_RB1:_F_RB1 + 4]
            rb2 = fblob[:, _F_RB2:_F_RB2 + 1]
            rb3 = fblob[0:1, _F_MISC:_F_MISC + 1]
            noise = fblob[0:1, _F_MISC + 1:_F_MISC + 2]
            epsb = fblob[0:1, _F_MISC + 2:_F_MISC + 3]
            ones_r = fblob[0:1, _F_ONESR:_F_ONESR + 128]

            mlp_tmp = []

            def mlp_layer(vec_in, w_sb, ktiles, ntiles, bias, act, nwidth=128):
                out_r = p_mlp.tile([128, max(ntiles, 1)], dt.float32,
                                   name=f"mlpv{len(mlp_tmp)}")
                mlp_tmp.append(out_r)
                ps = ps_m.tile([128, max(ntiles, 1)], dt.float32, tag="mlp",
                               name="mlpps")
                for t in range(ntiles):
                    for k in range(ktiles):
                        nc.tensor.matmul(
                            ps[:, t:t + 1],
                            w_sb[:, (k * ntiles + t) * nwidth:
                                 (k * ntiles + t) * nwidth + nwidth],
                            vec_in[:, k:k + 1],
                            start=(k == 0), stop=(k == ktiles - 1))
                for t in range(ntiles):
                    nc.scalar.activation(out_r[:, t:t + 1], ps[:, t:t + 1],
                                         act, bias=bias[:, t:t + 1])
                return out_r

            h1 = mlp_layer(featg, fe1, 1, 8, b1, AF.Silu)
            h2 = mlp_layer(h1, fe2, 8, 2, b2, AF.Identity)
            h3 = mlp_layer(h2, r1w, 2, 4, rb1, AF.Silu)
            h4 = mlp_layer(h3, r2w, 4, 1, rb2, AF.Silu)
            lps = ps_m.tile([1, 1], dt.float32, tag="mlp")
            nc.tensor.matmul(lps[:], r3w[:], h4[:], start=True, stop=True)
            logits = p_mlp.tile([1, 1], dt.float32)
            nc.scalar.activation(logits[:], lps[:], AF.Identity, bias=rb3)
            l1 = p_mlp.tile([1, 1], dt.float32)
            l2 = p_mlp.tile([1, 1], dt.float32)
            nc.scalar.activation(l1[:], noise, AF.Ln, bias=epsb)
            nc.scalar.activation(l2[:], l1[:], AF.Ln, bias=epsb, scale=-1.0)
            zin = p_mlp.tile([1, 1], dt.float32)
            nc.vector.tensor_tensor(zin[:], logits[:], l2[:], ALU.subtract)
            zsoft = p_mlp.tile([1, 1], dt.float32)
            nc.scalar.activation(zsoft[:], zin[:], AF.Sigmoid)
            zhard = p_mlp.tile([1, 1], dt.float32)
            nc.vector.tensor_scalar(zhard[:], zsoft[:], 0.5, None, ALU.is_gt)
            mps = ps_m.tile([128, 1], dt.float32, tag="mlp")
            nc.tensor.matmul(mps[:], ones_r, zhard[:], start=True, stop=True)
            nc.scalar.copy(mixb[:], mps[:])
            nc.vector.tensor_scalar(zbar[:], mixb[:], -1.0, 1.0, ALU.mult,
                                    ALU.add)
            nc.vector.tensor_scalar(negmix[:], mixb[:], -1.0, None, ALU.mult)

        # ---- phase 2: attention ------------------------------------------
        with ExitStack() as ph2:
            p_e = ph2.enter_context(tc.tile_pool(name="eband", bufs=2))
            p_tri = ph2.enter_context(tc.tile_pool(name="tri", bufs=2))
            p_cb = ph2.enter_context(tc.tile_pool(name="cmb", bufs=2))
            ps_sc = ph2.enter_context(
                tc.tile_pool(name="ps_sc", bufs=2, space="PSUM"))
            ps_os = ph2.enter_context(
                tc.tile_pool(name="ps_os", bufs=1, space="PSUM"))
            ps_om = ph2.enter_context(
                tc.tile_pool(name="ps_om", bufs=1, space="PSUM"))
            ps_sm = ph2.enter_context(
                tc.tile_pool(name="ps_sm", bufs=1, space="PSUM"))

            def acc_matmuls(dst_tile, ops, stationary):
                """Emit an accumulation group; ops = (J, lo, hi, src_ap).
                start=True on the first op touching each 128-col block."""
                written = set()
                for n, (J, lo, hi, src) in enumerate(ops):
                    blocks = set(range(lo // 128, hi // 128))
                    fresh = not (blocks & written)
                    assert fresh or blocks <= written, (n, ops)
                    written |= blocks
                    nc.tensor.matmul(
                        dst_tile[:, lo:hi] if dst_tile.shape[0] > 1
                        else dst_tile[0:1, lo:hi],
                        stationary(J), src,
                        start=fresh, stop=(n == len(ops) - 1),
                        skip_group_check=True)

            for h in range(HLOC):
                for c in range(NCH):
                    q0 = c * CH
                    nJ = 4 * c + 4          # key blocks 0..nJ-1
                    eT = p_e.tile([128, NBLK, CH], dt.bfloat16, tag="eT")
                    # masked copies for J = I-8 (I in chunk): 4 slots
                    etri = p_tri.tile([128, 2, 4, 128], dt.bfloat16,
                                      tag="etri")  # [mid|strm, slot]

                    # scores (transposed) + exp, two J blocks per psum tile
                    for J0 in range(0, nJ, 2):
                        sc = ps_sc.tile([128, 1024], dt.float32, tag="sc")
                        ws = []
                        for jj in range(2):
                            J = J0 + jj
                            lo = max(q0, J * 128)
                            w = (c + 1) * CH - lo
                            ws.append(w)
                            nc.tensor.matmul(
                                sc[:, jj * CH: jj * CH + w],
                                kT[:, J * 128:(J + 1) * 128],
                                qT[h][:, lo:lo + w],
                                start=True, stop=True)
                            if J >= 4 * c:  # diag block: causal mask
                                nc.vector.tensor_tensor(
                                    sc[:, jj * CH: jj * CH + 128],
                                    sc[:, jj * CH: jj * CH + 128],
                                    diagnegT[:], ALU.add)
                        if ws[0] == CH and ws[1] == CH:
                            nc.scalar.activation(
                                eT[:, J0:J0 + 2, :].rearrange(
                                    "p a b -> p (a b)"),
                                sc[:], AF.Exp, scale=SCALE)
                        else:
                            for jj in range(2):
                                J = J0 + jj
                                lo = max(q0, J * 128) - q0
                                nc.scalar.activation(
                                    eT[:, J, lo:CH],
                                    sc[:, jj * CH: jj * CH + ws[jj]],
                                    AF.Exp, scale=SCALE)

                    # triangle masks at J = I-8 for I in chunk (J>=1)
                    tslot = {}
                    for ii in range(4):
                        I = 4 * c + ii
                        J = I - 8
                        if J < 1:
                            continue
                        tslot[J] = ii
                        icol = I * 128 - q0
                        nc.vector.tensor_tensor(
                            etri[:, 0, ii, :], eT[:, J, icol:icol + 128],
                            trilow[:], ALU.mult)
                        nc.vector.tensor_tensor(
                            etri[:, 1, ii, :], eT[:, J, icol:icol + 128],
                            etri[:, 0, ii, :], ALU.subtract)

                    # op lists
                    full_ops = []
                    for J in range(nJ):
                        lo = max(q0, J * 128) - q0
                        full_ops.append((J, lo, CH, eT[:, J, lo:CH]))
                    mid_ops = []
                    for J in range(1, nJ):
                        ilo = max(4 * c, J + 9)
                        if ilo <= 4 * c + 3:
                            lo = ilo * 128 - q0
                            mid_ops.append((J, lo, CH, eT[:, J, lo:CH]))
                        if J in tslot:
                            t = tslot[J]
                            mid_ops.append(
                                (J, t * 128, t * 128 + 128, etri[:, 0, t, :]))
                    strm_ops = [(0, 0, CH, eT[:, 0, 0:CH])]   # sink
                    for J in range(max(1, 4 * c - 7), nJ):
                        lo = max(q0, J * 128) - q0
                        hi = min(CH, (J + 8) * 128 - q0)
                        strm_ops.append((J, lo, hi, eT[:, J, lo:hi]))
                        if J in tslot:
                            t = tslot[J]
                            strm_ops.append(
                                (J, t * 128, t * 128 + 128, etri[:, 1, t, :]))

                    # denominators (ones-vector matmuls) and PV accumulations
                    sums_f = ps_sm.tile([1, CH], dt.float32, tag="sf")
                    acc_matmuls(sums_f, full_ops, lambda J: oneskey)
                    o_s = ps_os.tile([128, CH], dt.float32, tag="os")
                    acc_matmuls(o_s, strm_ops,
                                lambda J: vN[:, J * 128:(J + 1) * 128])
                    if mid_ops:
                        sums_m = ps_sm.tile([1, CH], dt.float32, tag="sm")
                        acc_matmuls(sums_m, mid_ops, lambda J: oneskey)
                        o_m = ps_om.tile([128, CH], dt.float32, tag="om")
                        acc_matmuls(o_m, mid_ops,
                                    lambda J: vN[:, J * 128:(J + 1) * 128])

                    # combine + scale
                    scmb = p_cb.tile([1, CH], dt.float32, tag="scmb")
                    ocmb = p_cb.tile([128, CH], dt.float32, tag="ocmb")
                    if mid_ops:
                        # covered mid cols: [mlo, CH); others: strm == full
                        mlo = min(lo for _, lo, _, _ in mid_ops)
                        tmp = p_cb.tile([1, CH], dt.float32, tag="stmp")
                        nc.vector.tensor_scalar(
                            tmp[0:1, mlo:CH], sums_m[0:1, mlo:CH],
                            negmix[0:1, 0:1], None, ALU.mult)
                        if mlo > 0:
                            nc.vector.memset(tmp[0:1, 0:mlo], 0.0)
                        nc.vector.tensor_tensor(scmb[:], tmp[:], sums_f[:],
                                                ALU.add)
                        otmp = p_cb.tile([128, CH], dt.float32, tag="otmp")
                        nc.vector.tensor_scalar(
                            otmp[:, mlo:CH], o_m[:, mlo:CH], zbar[:, 0:1],
                            None, ALU.mult)
                        if mlo > 0:
                            nc.vector.memset(otmp[:, 0:mlo], 0.0)
                        nc.vector.tensor_tensor(ocmb[:], otmp[:], o_s[:],
                                                ALU.add)
                    else:
                        nc.vector.tensor_copy(scmb[:], sums_f[:])
                        nc.vector.tensor_copy(ocmb[:], o_s[:])
                    recip = p_cb.tile([1, CH], dt.float32, tag="recip")
                    nc.vector.reciprocal(recip[:], scmb[:])
                    rbc = p_cb.tile([128, CH], dt.float32, tag="rbc")
                    nc.gpsimd.partition_broadcast(rbc[:], recip[:])
                    osb = p_cb.tile([128, CH], dt.bfloat16, tag="osb")
                    nc.vector.tensor_tensor(osb[:], ocmb[:], rbc[:], ALU.mult)

                    # scatter to a2a bounce buffers (1 DMA per (h, c))
                    a2a_in = a2a_inA if h < 2 else a2a_inB
                    hh = h % 2
                    nc.sync.dma_start(
                        a2a_in[2 * c:2 * c + 2, hh].rearrange(
                            "q p r -> p q r"),
                        osb.rearrange("p (q r) -> p q r", r=ROWS))
                if h == 1:
                    nc.gpsimd.collective_compute(
                        "AllToAll", ALU.bypass,
                        replica_groups=[list(range(NCORES))],
                        ins=[a2a_inA.opt()], outs=[a2a_outA.opt()])
            nc.gpsimd.collective_compute(
                "AllToAll", ALU.bypass,
                replica_groups=[list(range(NCORES))],
                ins=[a2a_inB.opt()], outs=[a2a_outB.opt()])

        # ---- phase 3: output projection ----------------------------------
        with ExitStack() as ph3:
            p_oT = ph3.enter_context(tc.tile_pool(name="oT", bufs=1))
            p_wo = ph3.enter_context(tc.tile_pool(name="wo", bufs=4))
            p_os3 = ph3.enter_context(tc.tile_pool(name="outsb", bufs=2))
            ps_w = ph3.enter_context(
                tc.tile_pool(name="ps_w", bufs=1, space="PSUM"))

            # oT[k-tile g] = head (p, t) block; heads 0,1 from A, 2,3 from B
            oT = p_oT.tile([128, KT, ROWS], dt.bfloat16)
            for p in range(NCORES):
                nc.sync.dma_start(
                    oT[:, 4 * p:4 * p + 2, :],
                    a2a_outA[p].rearrange("t p r -> p t r"))
                nc.sync.dma_start(
                    oT[:, 4 * p + 2:4 * p + 4, :],
                    a2a_outB[p].rearrange("t p r -> p t r"))

            # contraction ordered: heads 0,1 of all peers first (from a2a A)
            G_ORDER = ([4 * p + t for p in range(NCORES) for t in (0, 1)]
                       + [4 * p + t for p in range(NCORES) for t in (2, 3)])
            for half in range(2):      # output column halves of 2048
                pso = [ps_w.tile([128, CH], dt.float32, tag=f"wo{i}",
                                 name=f"wo{i}") for i in range(8)]
                for gi, g in enumerate(G_ORDER):
                    wsl = p_wo.tile([128, 4 * CH], dt.bfloat16, tag="wo")
                    nc.sync.dma_start(
                        wsl[:], wo_d[g * 128:(g + 1) * 128,
                                     half * 2048:(half + 1) * 2048])
                    for ngi in range(4):
                        for st in range(2):
                            nc.tensor.matmul(
                                pso[ngi * 2 + st][:],
                                oT[:, g, st * 128:(st + 1) * 128],
                                wsl[:, ngi * CH:(ngi + 1) * CH],
                                start=(gi == 0), stop=(gi == KT - 1))
                for st in range(2):
                    osb = p_os3.tile([128, 4 * CH], dt.float32, tag="os")
                    for ngi in range(4):
                        nc.scalar.copy(osb[:, ngi * CH:(ngi + 1) * CH],
                                       pso[ngi * 2 + st][:])
                    nc.sync.dma_start(
                        out_d[st * 128:(st + 1) * 128,
                              half * 2048:(half + 1) * 2048], osb[:])

    nc.compile()
    return nc


_CACHE = {}


def _host_constants():
    inv = 10000.0 ** (-np.arange(0, D, 2, dtype=np.float64) / D)
    t = np.arange(S, dtype=np.float64)
    fr = np.outer(t, inv)                      # [S, 64]
    cos = np.cos(fr).T                         # [64, S]
    sin = np.sin(fr).T
    cos2 = np.vstack([cos, cos])
    sin2 = np.vstack([-sin, sin])
    a = np.arange(128)
    ident = np.eye(128, dtype=np.float32)
    diagnegT = np.where(a[:, None] <= a[None, :], 0.0, NEG).astype(np.float32)
    trilow = (a[:, None] <= a[None, :]).astype(np.float32)
    return cos2, sin2, ident, diagnegT, trilow


def kernel(hidden_states, Wq, Wk, Wv, Wo, fe1_w, fe1_b, fe2_w, fe2_b,
           r1_w, r1_b, r2_w, r2_b, r3_w, r3_b, router_noise):
    if "nc" not in _CACHE:
        _CACHE["nc"] = build()
    nc = _CACHE["nc"]

    hs = np.asarray(hidden_states, np.float32).reshape(S, HID)
    Wq = np.asarray(Wq, np.float32)
    Wk = np.asarray(Wk, np.float32)
    Wv = np.asarray(Wv, np.float32)
    Wo = np.asarray(Wo, np.float32)

    cos2, sin2, ident, diagnegT, trilow = _host_constants()

    # hs transposed, scaled, fp8-quantized, pair-tile layout
    hsT = np.ascontiguousarray(hs.T) * S_HS
    hsT8 = np.clip(hsT, -448, 448).astype(FP8)
    hsT8 = hsT8.reshape(KT, 128, S).transpose(1, 0, 2).copy()

    pool_idx = np.r_[0:POOL, S - POOL:S]
    hsp = (hs.T[:, pool_idx] * S_HS).reshape(KT, 128, 2 * POOL)
    hsp = hsp.transpose(1, 0, 2).reshape(128, KT * 2 * POOL)
    wqa = (Wq.reshape(HID, H, D).mean(axis=1) / S_HS).reshape(KT, 128, 128)
    wqa = wqa.transpose(1, 0, 2).reshape(128, KT * 128)

    # packed bf16 blob
    blob = np.zeros((128, _B_END), np.float64)
    blob[:, _B_IDENT:_B_IDENT + 128] = ident
    blob[:, _B_TRIL:_B_TRIL + 128] = trilow
    blob[:, _B_ONES:_B_ONES + 1] = 1.0
    blob[:, _B_COS:_B_COS + S] = cos2 * DEQ
    blob[:, _B_SIN:_B_SIN + S] = sin2 * DEQ
    blob[:, _B_COSP:_B_COSP + 2 * POOL] = cos2[:, pool_idx]
    blob[:, _B_SINP:_B_SINP + 2 * POOL] = sin2[:, pool_idx]
    blob[:, _B_WQA:_B_WQA + KT * 128] = wqa
    blob[:, _B_HSP:_B_HSP + KT * 2 * POOL] = hsp
    blob = blob.astype(BF16)

    def ktile_cols(w, ktiles):
        return np.concatenate(
            [w[k * 128:(k + 1) * 128, :] for k in range(ktiles)], axis=1)

    fblob = np.zeros((128, _F_END), np.float32)
    fblob[:, _F_DIAG:_F_DIAG + 128] = diagnegT
    fblob[:, _F_FE1:_F_FE1 + 1024] = np.asarray(fe1_w, np.float32)
    fblob[:, _F_FE2:_F_FE2 + 2048] = ktile_cols(np.asarray(fe2_w, np.float32), 8)
    fblob[:, _F_R1:_F_R1 + 1024] = ktile_cols(np.asarray(r1_w, np.float32), 2)
    fblob[:, _F_R2:_F_R2 + 512] = ktile_cols(np.asarray(r2_w, np.float32), 4)
    fblob[:, _F_R3:_F_R3 + 1] = np.asarray(r3_w, np.float32)
    fblob[:, _F_B1:_F_B1 + 8] = np.asarray(fe1_b, np.float32).reshape(8, 128).T
    fblob[:, _F_B2:_F_B2 + 2] = np.asarray(fe2_b, np.float32).reshape(2, 128).T
    fblob[:, _F_RB1:_F_RB1 + 4] = np.asarray(r1_b, np.float32).reshape(4, 128).T
    fblob[:, _F_RB2:_F_RB2 + 1] = np.asarray(r2_b, np.float32).reshape(1, 128).T
    fblob[0, _F_MISC + 0] = np.asarray(r3_b, np.float32).reshape(1)[0]
    fblob[0, _F_MISC + 1] = np.asarray(router_noise, np.float32).reshape(1)[0]
    fblob[0, _F_MISC + 2] = 1e-8
    fblob[0, _F_ONESR:_F_ONESR + 128] = 1.0

    wo_bf = np.ascontiguousarray(Wo.astype(BF16))

    in_maps = []
    for c in range(NCORES):
        wqkv = np.concatenate(
            [Wq[:, c * 512:(c + 1) * 512],
             Wk[:, c * 128:(c + 1) * 128],
             Wv[:, c * 128:(c + 1) * 128]], axis=1) * S_W
        wqkv8 = np.clip(wqkv, -448, 448).astype(FP8)
        wqkv8 = wqkv8.reshape(KT, 128, 768).transpose(1, 0, 2).copy()
        in_maps.append(dict(
            hsT8=hsT8.view(np.uint8), wqkv8=wqkv8.view(np.uint8),
            wo=wo_bf, blob=blob, fblob=fblob))

    res = run_bass_kernel_spmd(nc, in_maps, list(range(NCORES)))
    out = np.concatenate([res.results[c]["out_rows"] for c in range(NCORES)],
                         axis=0)
    return out.reshape(1, S, HID).astype(np.float32)


# revision 9
# speedup vs baseline: 1.9581x; 1.0568x over previous
"""Trainium2 Bass kernel for nn_LlamaAttention_61899068670751.

Sparse (streaming-LLM) attention layer, tensor-parallel over heads across 8
NeuronCores; core c owns q-heads [4c..4c+3] and kv-head c (GQA group = 4).

Key design points vs the v1 baseline:
  - hs is transposed + quantized to fp8e4 on the host; QKV projections run as
    fp8 DoubleRow matmuls (2 k-tiles per instruction, 0.5 cycles/row).
  - attention scores are computed TRANSPOSED (stationary = k block, moving =
    qT) so exp() output lands directly in the [key, query] layout needed by
    the PV matmul -- no per-block PE transposes and no PSUM->SBUF p copies.
  - o is accumulated as o_strm (sink+window mask) and o_mid (causal minus
    strm); softmax denominators via ones-vector matmuls; per-query scaling is
    applied once to oT (128 x S) instead of to p (S x S).
  - the tiny router MLP runs per-core from a replicated head-averaged Wq
    (rope commutes with the head average), eliminating the AllReduce.
  - o exchanged with two bf16 AllToAlls; output projection in bf16 with the
    contraction ordered so peers' heads 0-1 (first AllToAll) are consumed
    while the second AllToAll is still in flight.
  - DMas are batched aggressively (whole-chunk transfers, packed constant
    blobs) -- the HWDGE fixed cost (~625 ns per dma_start) dominates
    otherwise.
"""
import numpy as np
import ml_dtypes
from contextlib import ExitStack

import concourse.bacc as bacc
import concourse.mybir as mybir
import concourse.tile as tile
from concourse.bass_utils import run_bass_kernel_spmd

dt = mybir.dt
AF = mybir.ActivationFunctionType
ALU = mybir.AluOpType
AX = mybir.AxisListType
PM = mybir.MatmulPerfMode
BF16 = ml_dtypes.bfloat16
FP8 = ml_dtypes.float8_e4m3fn

NCORES = 8
S, H, KV, D, HID = 2048, 32, 8, 128, 4096
SINK, WIN, POOL = 128, 1024, 100
HLOC = H // NCORES          # 4 q heads per core
NBLK = S // 128             # 16 key/query blocks
NCH = 4                     # query chunks of 512
CH = 512
KT = HID // 128             # 32 contraction tiles
KP = KT // 2                # 16 fp8 pair-tiles
SCALE = 1.0 / float(np.sqrt(D))
NEG = -1.0e30
ROWS = S // NCORES          # 256 output rows per core

S_HS = 16.0                 # hs fp8 scale
S_W = 2048.0                # qkv weight fp8 scale
DEQ = 1.0 / (S_HS * S_W)    # per-operand dequant

# packed bf16 const blob column offsets
_B_IDENT = 0
_B_TRIL = 128
_B_ONES = 256
_B_COS = 257
_B_SIN = _B_COS + S
_B_COSP = _B_SIN + S
_B_SINP = _B_COSP + 2 * POOL
_B_WQA = _B_SINP + 2 * POOL
_B_HSP = _B_WQA + KT * 128
_B_END = _B_HSP + KT * 2 * POOL
# packed fp32 blob: diagnegT | mlp weights
_F_DIAG = 0
_F_FE1 = 128
_F_FE2 = _F_FE1 + 1024
_F_R1 = _F_FE2 + 2048
_F_R2 = _F_R1 + 1024
_F_R3 = _F_R2 + 512
_F_B1 = _F_R3 + 1
_F_B2 = _F_B1 + 8
_F_RB1 = _F_B2 + 2
_F_RB2 = _F_RB1 + 4
_F_MISC = _F_RB2 + 1        # [rb3, noise, eps] on partition 0
_F_ONESR = _F_MISC + 3      # [1, 128] ones row on partition 0
_F_END = _F_ONESR + 128


def build():
    nc = bacc.Bacc("TRN2", target_bir_lowering=False, debug=False,
                   num_devices=NCORES)

    def din(name, shape, d):
        return nc.dram_tensor(name, shape, d, kind="ExternalInput").ap()

    hsT8_d = din("hsT8", [128, KT, S], dt.float8e4)
    wqkv8_d = din("wqkv8", [128, KT, 768], dt.float8e4)
    wo_d = din("wo", [HID, HID], dt.bfloat16)
    blob_d = din("blob", [128, _B_END], dt.bfloat16)
    fblob_d = din("fblob", [128, _F_END], dt.float32)

    out_d = nc.dram_tensor("out_rows", [ROWS, HID], dt.float32,
                           kind="ExternalOutput").ap()

    with tile.TileContext(nc) as tc, ExitStack() as top:
        const = top.enter_context(tc.tile_pool(name="const", bufs=1))
        persist = top.enter_context(tc.tile_pool(name="persist", bufs=1))
        dram = top.enter_context(tc.tile_pool(name="dram", bufs=1, space="DRAM"))

        blob = const.tile([128, _B_END], dt.bfloat16)
        fblob = const.tile([128, _F_END], dt.float32)
        ident = blob[:, _B_IDENT:_B_IDENT + 128]
        trilow = blob[:, _B_TRIL:_B_TRIL + 128]
        oneskey = blob[:, _B_ONES:_B_ONES + 1]
        cos2 = blob[:, _B_COS:_B_COS + S]
        sin2 = blob[:, _B_SIN:_B_SIN + S]
        cosp = blob[:, _B_COSP:_B_COSP + 2 * POOL]
        sinp = blob[:, _B_SINP:_B_SINP + 2 * POOL]
        wqa = blob[:, _B_WQA:_B_WQA + KT * 128].rearrange(
            "p (k f) -> p k f", f=128)
        hsp = blob[:, _B_HSP:_B_HSP + KT * 2 * POOL].rearrange(
            "p (k f) -> p k f", f=2 * POOL)
        diagnegT = fblob[:, _F_DIAG:_F_DIAG + 128]

        qT = [persist.tile([128, S], dt.bfloat16, name=f"qT{h}", tag=f"qT{h}")
              for h in range(HLOC)]
        kT = persist.tile([128, S], dt.bfloat16)
        vN = persist.tile([128, S], dt.bfloat16)    # v natural, 16 key blocks
        mixb = persist.tile([128, 1], dt.float32)   # z broadcast
        zbar = persist.tile([128, 1], dt.float32)   # 1-z
        negmix = persist.tile([128, 1], dt.float32)  # -z

        # a2a bounce: one exchange per local head
        a2a_in = [dram.tile([NCORES, 128, ROWS], dt.bfloat16,
                            name=f"a2ai{h}") for h in range(HLOC)]
        a2a_out = [dram.tile([NCORES, 128, ROWS], dt.bfloat16,
                             name=f"a2ao{h}") for h in range(HLOC)]

        # ---- phase 1: QKV fp8 DoubleRow + rope + router feature ----------
        with ExitStack() as ph1:
            p_w8 = ph1.enter_context(tc.tile_pool(name="w8", bufs=1))
            p_hs8 = ph1.enter_context(tc.tile_pool(name="hs8", bufs=2))
            p_rope = ph1.enter_context(tc.tile_pool(name="rope", bufs=2))
            p_rsb = ph1.enter_context(tc.tile_pool(name="rsb", bufs=1))
            ps_acc = ph1.enter_context(
                tc.tile_pool(name="ps_acc", bufs=1, space="PSUM"))
            ps_tr = ph1.enter_context(
                tc.tile_pool(name="ps_tr", bufs=1, space="PSUM"))
            ps_rt = ph1.enter_context(
                tc.tile_pool(name="ps_rt", bufs=1, space="PSUM"))

            # all qkv weights resident (24 KiB/partition); split DMA so
            # the first pair-tiles land fast, then chunk-0 hs, then blobs
            w8 = p_w8.tile([128, KT, 768], dt.float8e4)
            nc.sync.dma_start(w8[:, 0:8], wqkv8_d[:, 0:8])
            hs8_tiles = [p_hs8.tile([128, KT, CH], dt.float8e4, tag="hs8",
                                    name=f"hs8_{g}") for g in range(NCH)]
            nc.sync.dma_start(hs8_tiles[0][:], hsT8_d[:, :, 0:CH])
            nc.sync.dma_start(w8[:, 8:KT], wqkv8_d[:, 8:KT])
            nc.sync.dma_start(blob[:], blob_d[:])
            nc.sync.dma_start(fblob[:], fblob_d[:])

            rt_ps = ps_rt.tile([128, 2 * POOL], dt.float32)

            for g in range(NCH):
                s0 = g * CH
                accs = [ps_acc.tile([128, CH], dt.float32, tag=f"acc{i}",
                                    name=f"acc{i}") for i in range(6)]
                hs8 = hs8_tiles[g]
                if g > 0:
                    nc.sync.dma_start(hs8[:], hsT8_d[:, :, s0:s0 + CH])
                for t in range(KP):
                    for i in range(6):
                        nc.tensor.matmul(
                            accs[i][:],
                            w8[:, 2 * t:2 * t + 2, i * 128:(i + 1) * 128],
                            hs8[:, 2 * t:2 * t + 2, :],
                            start=(t == 0), stop=(t == KP - 1),
                            perf_mode=PM.DoubleRow)
                if g == 0:
                    # router: q_avgT = sum_k wqa[k].T @ hsp[k]
                    for k in range(KT):
                        nc.tensor.matmul(rt_ps[:], wqa[:, k], hsp[:, k],
                                         start=(k == 0), stop=(k == KT - 1))

                # rope for q heads (0..3) and k (4); cos2/sin2 carry dequant
                lin = p_rope.tile([128, 5, CH], dt.bfloat16, tag="lin")
                rot = p_rope.tile([128, 5, CH], dt.bfloat16, tag="rot")
                for i in range(5):
                    nc.scalar.copy(lin[:, i], accs[i][:])
                lin2 = lin.rearrange("p a b -> p (a b)")
                rot2 = rot.rearrange("p a b -> p (a b)")
                nc.sync.dma_start(rot2[0:64, :], lin2[64:128, :])
                nc.sync.dma_start(rot2[64:128, :], lin2[0:64, :])
                for i in range(5):
                    dest = qT[i] if i < HLOC else kT
                    t1 = p_rope.tile([128, CH], dt.bfloat16, tag="t1")
                    nc.vector.tensor_tensor(t1[:], lin[:, i],
                                            cos2[:, s0:s0 + CH], ALU.mult)
                    nc.vector.tensor_tensor(rot[:, i], rot[:, i],
                                            sin2[:, s0:s0 + CH], ALU.mult)
                    nc.vector.tensor_tensor(dest[:, s0:s0 + CH], t1[:],
                                            rot[:, i], ALU.add)
                # v: dequant copy then transpose to natural layout
                vT = p_rope.tile([128, CH], dt.bfloat16, tag="vT")
                nc.scalar.activation(vT[:], accs[5][:], AF.Copy, scale=DEQ)
                ptr = ps_tr.tile([128, CH], dt.bfloat16, tag="tr")
                for ss in range(4):
                    nc.tensor.transpose(ptr[:, ss * 128:(ss + 1) * 128],
                                        vT[:, ss * 128:(ss + 1) * 128],
                                        ident[:])
                nc.vector.tensor_copy(vN[:, s0:s0 + CH], ptr[:])

            # router rope + feature (q_avg is true-scaled: hsp x16, wqa /16)
            rlin = p_rsb.tile([128, 2 * POOL], dt.bfloat16)
            rrot = p_rsb.tile([128, 2 * POOL], dt.bfloat16)
            rt1 = p_rsb.tile([128, 2 * POOL], dt.bfloat16)
            nc.scalar.copy(rlin[:], rt_ps[:])
            nc.sync.dma_start(rrot[0:64, :], rlin[64:128, :])
            nc.sync.dma_start(rrot[64:128, :], rlin[0:64, :])
            nc.vector.tensor_tensor(rt1[:], rlin[:], cosp[:], ALU.mult)
            nc.vector.tensor_tensor(rrot[:], rrot[:], sinp[:], ALU.mult)
            nc.vector.tensor_tensor(rt1[:], rt1[:], rrot[:], ALU.add)
            feat = p_rsb.tile([128, 1], dt.float32)
            nc.vector.tensor_reduce(feat[:], rt1[:], AX.X, ALU.add)
            featg = persist.tile([128, 1], dt.float32)
            nc.scalar.activation(featg[:], feat[:], AF.Copy,
                                 scale=1.0 / (2 * POOL))

        # ---- router MLP (tiny, replicated) -------------------------------
        with ExitStack() as phm:
            p_mlp = phm.enter_context(tc.tile_pool(name="mlp", bufs=1))
            ps_m = phm.enter_context(
                tc.tile_pool(name="ps_m", bufs=1, space="PSUM"))

            fe1 = fblob[:, _F_FE1:_F_FE1 + 1024]
            fe2 = fblob[:, _F_FE2:_F_FE2 + 2048]
            r1w = fblob[:, _F_R1:_F_R1 + 1024]
            r2w = fblob[:, _F_R2:_F_R2 + 512]
            r3w = fblob[:, _F_R3:_F_R3 + 1]
            b1 = fblob[:, _F_B1:_F_B1 + 8]
            b2 = fblob[:, _F_B2:_F_B2 + 2]
            rb1 = fblob[:, _F_RB1:_F_RB1 + 4]
            rb2 = fblob[:, _F_RB2:_F_RB2 + 1]
            rb3 = fblob[0:1, _F_MISC:_F_MISC + 1]
            noise = fblob[0:1, _F_MISC + 1:_F_MISC + 2]
            epsb = fblob[0:1, _F_MISC + 2:_F_MISC + 3]
            ones_r = fblob[0:1, _F_ONESR:_F_ONESR + 128]

            mlp_tmp = []

            def mlp_layer(vec_in, w_sb, ktiles, ntiles, bias, act, nwidth=128):
                out_r = p_mlp.tile([128, max(ntiles, 1)], dt.float32,
                                   name=f"mlpv{len(mlp_tmp)}")
                mlp_tmp.append(out_r)
                ps = ps_m.tile([128, max(ntiles, 1)], dt.float32, tag="mlp",
                               name="mlpps")
                for t in range(ntiles):
                    for k in range(ktiles):
                        nc.tensor.matmul(
                            ps[:, t:t + 1],
                            w_sb[:, (k * ntiles + t) * nwidth:
                                 (k * ntiles + t) * nwidth + nwidth],
                            vec_in[:, k:k + 1],
                            start=(k == 0), stop=(k == ktiles - 1))
                for t in range(ntiles):
                    nc.scalar.activation(out_r[:, t:t + 1], ps[:, t:t + 1],
                                         act, bias=bias[:, t:t + 1])
                return out_r

            h1 = mlp_layer(featg, fe1, 1, 8, b1, AF.Silu)
            h2 = mlp_layer(h1, fe2, 8, 2, b2, AF.Identity)
            h3 = mlp_layer(h2, r1w, 2, 4, rb1, AF.Silu)
            h4 = mlp_layer(h3, r2w, 4, 1, rb2, AF.Silu)
            lps = ps_m.tile([1, 1], dt.float32, tag="mlp")
            nc.tensor.matmul(lps[:], r3w[:], h4[:], start=True, stop=True)
            logits = p_mlp.tile([1, 1], dt.float32)
            nc.scalar.activation(logits[:], lps[:], AF.Identity, bias=rb3)
            l1 = p_mlp.tile([1, 1], dt.float32)
            l2 = p_mlp.tile([1, 1], dt.float32)
            nc.scalar.activation(l1[:], noise, AF.Ln, bias=epsb)
            nc.scalar.activation(l2[:], l1[:], AF.Ln, bias=epsb, scale=-1.0)
            zin = p_mlp.tile([1, 1], dt.float32)
            nc.vector.tensor_tensor(zin[:], logits[:], l2[:], ALU.subtract)
            zsoft = p_mlp.tile([1, 1], dt.float32)
            nc.scalar.activation(zsoft[:], zin[:], AF.Sigmoid)
            zhard = p_mlp.tile([1, 1], dt.float32)
            nc.vector.tensor_scalar(zhard[:], zsoft[:], 0.5, None, ALU.is_gt)
            mps = ps_m.tile([128, 1], dt.float32, tag="mlp")
            nc.tensor.matmul(mps[:], ones_r, zhard[:], start=True, stop=True)
            nc.scalar.copy(mixb[:], mps[:])
            nc.vector.tensor_scalar(zbar[:], mixb[:], -1.0, 1.0, ALU.mult,
                                    ALU.add)
            nc.vector.tensor_scalar(negmix[:], mixb[:], -1.0, None, ALU.mult)

        # ---- phase 2: attention ------------------------------------------
        with ExitStack() as ph2:
            p_e = ph2.enter_context(tc.tile_pool(name="eband", bufs=2))
            p_tri = ph2.enter_context(tc.tile_pool(name="tri", bufs=2))
            p_cb = ph2.enter_context(tc.tile_pool(name="cmb", bufs=2))
            ps_sc = ph2.enter_context(
                tc.tile_pool(name="ps_sc", bufs=2, space="PSUM"))
            ps_os = ph2.enter_context(
                tc.tile_pool(name="ps_os", bufs=1, space="PSUM"))
            ps_om = ph2.enter_context(
                tc.tile_pool(name="ps_om", bufs=1, space="PSUM"))
            ps_sm = ph2.enter_context(
                tc.tile_pool(name="ps_sm", bufs=1, space="PSUM"))

            def acc_matmuls(dst_tile, ops, stationary):
                """Emit an accumulation group; ops = (J, lo, hi, src_ap).
                start=True on the first op touching each 128-col block."""
                written = set()
                for n, (J, lo, hi, src) in enumerate(ops):
                    blocks = set(range(lo // 128, hi // 128))
                    fresh = not (blocks & written)
                    assert fresh or blocks <= written, (n, ops)
                    written |= blocks
                    nc.tensor.matmul(
                        dst_tile[:, lo:hi] if dst_tile.shape[0] > 1
                        else dst_tile[0:1, lo:hi],
                        stationary(J), src,
                        start=fresh, stop=(n == len(ops) - 1),
                        skip_group_check=True)

            for h in range(HLOC):
                for c in range(NCH):
                    q0 = c * CH
                    nJ = 4 * c + 4          # key blocks 0..nJ-1
                    eT = p_e.tile([128, NBLK, CH], dt.bfloat16, tag="eT")
                    # masked copies for J = I-8 (I in chunk): 4 slots
                    etri = p_tri.tile([128, 2, 4, 128], dt.bfloat16,
                                      tag="etri")  # [mid|strm, slot]

                    # scores (transposed) + exp, two J blocks per psum tile
                    for J0 in range(0, nJ, 2):
                        sc = ps_sc.tile([128, 1024], dt.float32, tag="sc")
                        ws = []
                        for jj in range(2):
                            J = J0 + jj
                            lo = max(q0, J * 128)
                            w = (c + 1) * CH - lo
                            ws.append(w)
                            nc.tensor.matmul(
                                sc[:, jj * CH: jj * CH + w],
                                kT[:, J * 128:(J + 1) * 128],
                                qT[h][:, lo:lo + w],
                                start=True, stop=True)
                            if J >= 4 * c:  # diag block: causal mask
                                nc.vector.tensor_tensor(
                                    sc[:, jj * CH: jj * CH + 128],
                                    sc[:, jj * CH: jj * CH + 128],
                                    diagnegT[:], ALU.add)
                        if ws[0] == CH and ws[1] == CH:
                            nc.scalar.activation(
                                eT[:, J0:J0 + 2, :].rearrange(
                                    "p a b -> p (a b)"),
                                sc[:], AF.Exp, scale=SCALE)
                        else:
                            for jj in range(2):
                                J = J0 + jj
                                lo = max(q0, J * 128) - q0
                                nc.scalar.activation(
                                    eT[:, J, lo:CH],
                                    sc[:, jj * CH: jj * CH + ws[jj]],
                                    AF.Exp, scale=SCALE)

                    # triangle masks at J = I-8 for I in chunk (J>=1)
                    tslot = {}
                    for ii in range(4):
                        I = 4 * c + ii
                        J = I - 8
                        if J < 1:
                            continue
                        tslot[J] = ii
                        icol = I * 128 - q0
                        nc.vector.tensor_tensor(
                            etri[:, 0, ii, :], eT[:, J, icol:icol + 128],
                            trilow[:], ALU.mult)
                        nc.vector.tensor_tensor(
                            etri[:, 1, ii, :], eT[:, J, icol:icol + 128],
                            etri[:, 0, ii, :], ALU.subtract)

                    # op lists
                    full_ops = []
                    for J in range(nJ):
                        lo = max(q0, J * 128) - q0
                        full_ops.append((J, lo, CH, eT[:, J, lo:CH]))
                    mid_ops = []
                    for J in range(1, nJ):
                        ilo = max(4 * c, J + 9)
                        if ilo <= 4 * c + 3:
                            lo = ilo * 128 - q0
                            mid_ops.append((J, lo, CH, eT[:, J, lo:CH]))
                        if J in tslot:
                            t = tslot[J]
                            mid_ops.append(
                                (J, t * 128, t * 128 + 128, etri[:, 0, t, :]))
                    strm_ops = [(0, 0, CH, eT[:, 0, 0:CH])]   # sink
                    for J in range(max(1, 4 * c - 7), nJ):
                        lo = max(q0, J * 128) - q0
                        hi = min(CH, (J + 8) * 128 - q0)
                        strm_ops.append((J, lo, hi, eT[:, J, lo:hi]))
                        if J in tslot:
                            t = tslot[J]
                            strm_ops.append(
                                (J, t * 128, t * 128 + 128, etri[:, 1, t, :]))

                    # denominators (ones-vector matmuls) and PV accumulations
                    sums_f = ps_sm.tile([1, CH], dt.float32, tag="sf")
                    acc_matmuls(sums_f, full_ops, lambda J: oneskey)
                    o_s = ps_os.tile([128, CH], dt.float32, tag="os")
                    acc_matmuls(o_s, strm_ops,
                                lambda J: vN[:, J * 128:(J + 1) * 128])
                    if mid_ops:
                        sums_m = ps_sm.tile([1, CH], dt.float32, tag="sm")
                        acc_matmuls(sums_m, mid_ops, lambda J: oneskey)
                        o_m = ps_om.tile([128, CH], dt.float32, tag="om")
                        acc_matmuls(o_m, mid_ops,
                                    lambda J: vN[:, J * 128:(J + 1) * 128])

                    # combine + scale
                    scmb = p_cb.tile([1, CH], dt.float32, tag="scmb")
                    ocmb = p_cb.tile([128, CH], dt.float32, tag="ocmb")
                    if mid_ops:
                        # covered mid cols: [mlo, CH); others: strm == full
                        mlo = min(lo for _, lo, _, _ in mid_ops)
                        tmp = p_cb.tile([1, CH], dt.float32, tag="stmp")
                        nc.vector.tensor_scalar(
                            tmp[0:1, mlo:CH], sums_m[0:1, mlo:CH],
                            negmix[0:1, 0:1], None, ALU.mult)
                        if mlo > 0:
                            nc.vector.memset(tmp[0:1, 0:mlo], 0.0)
                        nc.vector.tensor_tensor(scmb[:], tmp[:], sums_f[:],
                                                ALU.add)
                        otmp = p_cb.tile([128, CH], dt.float32, tag="otmp")
                        nc.vector.tensor_scalar(
                            otmp[:, mlo:CH], o_m[:, mlo:CH], zbar[:, 0:1],
                            None, ALU.mult)
                        if mlo > 0:
                            nc.vector.memset(otmp[:, 0:mlo], 0.0)
                        nc.vector.tensor_tensor(ocmb[:], otmp[:], o_s[:],
                                                ALU.add)
                    else:
                        nc.vector.tensor_copy(scmb[:], sums_f[:])
                        nc.vector.tensor_copy(ocmb[:], o_s[:])
                    recip = p_cb.tile([1, CH], dt.float32, tag="recip")
                    nc.vector.reciprocal(recip[:], scmb[:])
                    rbc = p_cb.tile([128, CH], dt.float32, tag="rbc")
                    nc.gpsimd.partition_broadcast(rbc[:], recip[:])
                    osb = p_cb.tile([128, CH], dt.bfloat16, tag="osb")
                    nc.vector.tensor_tensor(osb[:], ocmb[:], rbc[:], ALU.mult)

                    # scatter to a2a bounce buffers (1 DMA per (h, c))
                    nc.sync.dma_start(
                        a2a_in[h][2 * c:2 * c + 2].rearrange(
                            "q p r -> p q r"),
                        osb.rearrange("p (q r) -> p q r", r=ROWS))
                nc.gpsimd.collective_compute(
                    "AllToAll", ALU.bypass,
                    replica_groups=[list(range(NCORES))],
                    ins=[a2a_in[h].opt()], outs=[a2a_out[h].opt()])

        # ---- phase 3: output projection ----------------------------------
        with ExitStack() as ph3:
            p_oT = ph3.enter_context(tc.tile_pool(name="oT", bufs=1))
            p_wo = ph3.enter_context(tc.tile_pool(name="wo", bufs=4))
            p_os3 = ph3.enter_context(tc.tile_pool(name="outsb", bufs=2))
            ps_w = ph3.enter_context(
                tc.tile_pool(name="ps_w", bufs=1, space="PSUM"))

            # oT[k-tile g] = head (p, t) block; one gather per a2a
            oT = p_oT.tile([128, KT, ROWS], dt.bfloat16)
            for t in range(HLOC):
                nc.sync.dma_start(
                    oT[:, t::HLOC, :],
                    a2a_out[t].rearrange("q p r -> p q r"))

            # contraction ordered by head so a2a h arrives just in time
            G_ORDER = [4 * p + t for t in range(HLOC) for p in range(NCORES)]
            for half in range(2):      # output column halves of 2048
                pso = [ps_w.tile([128, CH], dt.float32, tag=f"wo{i}",
                                 name=f"wo{i}") for i in range(8)]
                for gi, g in enumerate(G_ORDER):
                    wsl = p_wo.tile([128, 4 * CH], dt.bfloat16, tag="wo")
                    nc.sync.dma_start(
                        wsl[:], wo_d[g * 128:(g + 1) * 128,
                                     half * 2048:(half + 1) * 2048])
                    for ngi in range(4):
                        for st in range(2):
                            nc.tensor.matmul(
                                pso[ngi * 2 + st][:],
                                oT[:, g, st * 128:(st + 1) * 128],
                                wsl[:, ngi * CH:(ngi + 1) * CH],
                                start=(gi == 0), stop=(gi == KT - 1))
                for st in range(2):
                    osb = p_os3.tile([128, 4 * CH], dt.float32, tag="os")
                    for ngi in range(4):
                        nc.scalar.copy(osb[:, ngi * CH:(ngi + 1) * CH],
                                       pso[ngi * 2 + st][:])
                    nc.sync.dma_start(
                        out_d[st * 128:(st + 1) * 128,
                              half * 2048:(half + 1) * 2048], osb[:])

    nc.compile()
    return nc


_CACHE = {}


def _host_constants():
    inv = 10000.0 ** (-np.arange(0, D, 2, dtype=np.float64) / D)
    t = np.arange(S, dtype=np.float64)
    fr = np.outer(t, inv)                      # [S, 64]
    cos = np.cos(fr).T                         # [64, S]
    sin = np.sin(fr).T
    cos2 = np.vstack([cos, cos])
    sin2 = np.vstack([-sin, sin])
    a = np.arange(128)
    ident = np.eye(128, dtype=np.float32)
    diagnegT = np.where(a[:, None] <= a[None, :], 0.0, NEG).astype(np.float32)
    trilow = (a[:, None] <= a[None, :]).astype(np.float32)
    return cos2, sin2, ident, diagnegT, trilow


def kernel(hidden_states, Wq, Wk, Wv, Wo, fe1_w, fe1_b, fe2_w, fe2_b,
           r1_w, r1_b, r2_w, r2_b, r3_w, r3_b, router_noise):
    if "nc" not in _CACHE:
        _CACHE["nc"] = build()
    nc = _CACHE["nc"]

    hs = np.asarray(hidden_states, np.float32).reshape(S, HID)
    Wq = np.asarray(Wq, np.float32)
    Wk = np.asarray(Wk, np.float32)
    Wv = np.asarray(Wv, np.float32)
    Wo = np.asarray(Wo, np.float32)

    cos2, sin2, ident, diagnegT, trilow = _host_constants()

    # hs transposed, scaled, fp8-quantized, pair-tile layout
    hsT = np.ascontiguousarray(hs.T) * S_HS
    hsT8 = np.clip(hsT, -448, 448).astype(FP8)
    hsT8 = hsT8.reshape(KT, 128, S).transpose(1, 0, 2).copy()

    pool_idx = np.r_[0:POOL, S - POOL:S]
    hsp = (hs.T[:, pool_idx] * S_HS).reshape(KT, 128, 2 * POOL)
    hsp = hsp.transpose(1, 0, 2).reshape(128, KT * 2 * POOL)
    wqa = (Wq.reshape(HID, H, D).mean(axis=1) / S_HS).reshape(KT, 128, 128)
    wqa = wqa.transpose(1, 0, 2).reshape(128, KT * 128)

    # packed bf16 blob
    blob = np.zeros((128, _B_END), np.float64)
    blob[:, _B_IDENT:_B_IDENT + 128] = ident
    blob[:, _B_TRIL:_B_TRIL + 128] = trilow
    blob[:, _B_ONES:_B_ONES + 1] = 1.0
    blob[:, _B_COS:_B_COS + S] = cos2 * DEQ
    blob[:, _B_SIN:_B_SIN + S] = sin2 * DEQ
    blob[:, _B_COSP:_B_COSP + 2 * POOL] = cos2[:, pool_idx]
    blob[:, _B_SINP:_B_SINP + 2 * POOL] = sin2[:, pool_idx]
    blob[:, _B_WQA:_B_WQA + KT * 128] = wqa
    blob[:, _B_HSP:_B_HSP + KT * 2 * POOL] = hsp
    blob = blob.astype(BF16)

    def ktile_cols(w, ktiles):
        return np.concatenate(
            [w[k * 128:(k + 1) * 128, :] for k in range(ktiles)], axis=1)

    fblob = np.zeros((128, _F_END), np.float32)
    fblob[:, _F_DIAG:_F_DIAG + 128] = diagnegT
    fblob[:, _F_FE1:_F_FE1 + 1024] = np.asarray(fe1_w, np.float32)
    fblob[:, _F_FE2:_F_FE2 + 2048] = ktile_cols(np.asarray(fe2_w, np.float32), 8)
    fblob[:, _F_R1:_F_R1 + 1024] = ktile_cols(np.asarray(r1_w, np.float32), 2)
    fblob[:, _F_R2:_F_R2 + 512] = ktile_cols(np.asarray(r2_w, np.float32), 4)
    fblob[:, _F_R3:_F_R3 + 1] = np.asarray(r3_w, np.float32)
    fblob[:, _F_B1:_F_B1 + 8] = np.asarray(fe1_b, np.float32).reshape(8, 128).T
    fblob[:, _F_B2:_F_B2 + 2] = np.asarray(fe2_b, np.float32).reshape(2, 128).T
    fblob[:, _F_RB1:_F_RB1 + 4] = np.asarray(r1_b, np.float32).reshape(4, 128).T
    fblob[:, _F_RB2:_F_RB2 + 1] = np.asarray(r2_b, np.float32).reshape(1, 128).T
    fblob[0, _F_MISC + 0] = np.asarray(r3_b, np.float32).reshape(1)[0]
    fblob[0, _F_MISC + 1] = np.asarray(router_noise, np.float32).reshape(1)[0]
    fblob[0, _F_MISC + 2] = 1e-8
    fblob[0, _F_ONESR:_F_ONESR + 128] = 1.0

    wo_bf = np.ascontiguousarray(Wo.astype(BF16))

    in_maps = []
    for c in range(NCORES):
        wqkv = np.concatenate(
            [Wq[:, c * 512:(c + 1) * 512],
             Wk[:, c * 128:(c + 1) * 128],
             Wv[:, c * 128:(c + 1) * 128]], axis=1) * S_W
        wqkv8 = np.clip(wqkv, -448, 448).astype(FP8)
        wqkv8 = wqkv8.reshape(KT, 128, 768).transpose(1, 0, 2).copy()
        in_maps.append(dict(
            hsT8=hsT8.view(np.uint8), wqkv8=wqkv8.view(np.uint8),
            wo=wo_bf, blob=blob, fblob=fblob))

    res = run_bass_kernel_spmd(nc, in_maps, list(range(NCORES)))
    out = np.concatenate([res.results[c]["out_rows"] for c in range(NCORES)],
                         axis=0)
    return out.reshape(1, S, HID).astype(np.float32)
